# revision 1
# baseline (speedup 1.0000x reference)
"""Mamba-2-layer net on 8 trn2 NeuronCores.

Sharding: core c -> batch b = c // 4, d_inner quarter q = c % 4 (256 channels).
Everything feature-major [channel partitions, time free].  The selective scan
runs as nc.vector.tensor_tensor_scan along the free (time) axis.  One bf16
AllReduce per layer (out_proj partials) within each 4-core group.
"""

import os
import sys
import numpy as np

sys.path.insert(0, "/opt/trn_rl_repo")

import concourse.bass as bass
import concourse.bacc as bacc
import concourse.tile as tile
import concourse.mybir as mybir
from concourse.bass_utils import run_bass_kernel_spmd

dt = mybir.dt
AF = mybir.ActivationFunctionType
OP = mybir.AluOpType

# model dims
B, L = 2, 2048
IN_DIM = 16
D_MODEL = 512
D_INNER = 1024
D_STATE = 16
D_CONV = 4
DT_RANK = 32
N_LAYERS = 2
EPS = 1e-5

# sharding
N_CORES = 8
QUART = D_INNER // 4          # 256 channels per core
T = L                         # tokens per core (one batch)
P = 128
NCH = T // 512                # psum chunks of 512
JT = QUART // P               # 2 d-tiles per core quarter
GX = D_INNER // P             # 8 xin tiles (full, replicated in group)
KM = D_MODEL // P             # 4 k-tiles over d_model
PAD = 4                       # left pad for causal conv

_CACHE = {}


def _build_program(reps=1, use_cc=True, single_core=False):
    key = ("prog", reps, use_cc, single_core)
    if key in _CACHE:
        return _CACHE[key]

    nc = bacc.Bacc(
        "TRN2",
        target_bir_lowering=False,
        debug=False,
        enable_asserts=False,
        num_devices=1 if single_core else N_CORES,
    )

    bf = dt.bfloat16
    f32 = dt.float32

    # ---------------- DRAM I/O ----------------
    xT = nc.dram_tensor("xT", [IN_DIM, T], bf, kind="ExternalInput").ap()
    lin1T = nc.dram_tensor("lin1T", [IN_DIM, D_MODEL], bf, kind="ExternalInput").ap()
    lin1b = nc.dram_tensor("lin1b", [P, KM], f32, kind="ExternalInput").ap()
    lin2Tp = nc.dram_tensor("lin2Tp", [P, KM], bf, kind="ExternalInput").ap()
    lin2b = nc.dram_tensor("lin2b", [1, 1], f32, kind="ExternalInput").ap()
    idn_d = nc.dram_tensor("idn", [P, P], bf, kind="ExternalInput").ap()

    ipx_d, ipz_d, convw_d, convb_d, xp_d, dtw_d, dtb_d, asc_d, dp_d, op_d = (
        [], [], [], [], [], [], [], [], [], [])
    for l in range(N_LAYERS):
        ipx_d.append(nc.dram_tensor(f"ipx{l}", [D_MODEL, D_INNER], bf, kind="ExternalInput").ap())
        ipz_d.append(nc.dram_tensor(f"ipz{l}", [D_MODEL, QUART], bf, kind="ExternalInput").ap())
        convw_d.append(nc.dram_tensor(f"convw{l}", [P, GX * D_CONV], f32, kind="ExternalInput").ap())
        convb_d.append(nc.dram_tensor(f"convb{l}", [P, GX], f32, kind="ExternalInput").ap())
        xp_d.append(nc.dram_tensor(f"xp{l}", [D_INNER, DT_RANK + 2 * D_STATE], bf, kind="ExternalInput").ap())
        dtw_d.append(nc.dram_tensor(f"dtw{l}", [DT_RANK, QUART], bf, kind="ExternalInput").ap())
        dtb_d.append(nc.dram_tensor(f"dtb{l}", [P, JT], f32, kind="ExternalInput").ap())
        asc_d.append(nc.dram_tensor(f"asc{l}", [P, JT * D_STATE], f32, kind="ExternalInput").ap())
        dp_d.append(nc.dram_tensor(f"dp{l}", [P, JT], f32, kind="ExternalInput").ap())
        op_d.append(nc.dram_tensor(f"op{l}", [QUART, D_MODEL], bf, kind="ExternalInput").ap())

    w2q_d = nc.dram_tensor("w2q", [QUART, 1], bf, kind="ExternalInput").ap()
    yrow_d = nc.dram_tensor("yrow", [1, T], f32, kind="ExternalOutput").ap()

    with tile.TileContext(nc) as tc:
        with (
            tc.tile_pool(name="wpool", bufs=1) as wp,
            tc.tile_pool(name="hpool", bufs=1) as hp,
            tc.tile_pool(name="dram", bufs=1, space="DRAM") as dramp,
        ):
            # ---------------- load weights ----------------
            xT_s = wp.tile([IN_DIM, T], bf, tag="xT", name="xT")
            nc.gpsimd.dma_start(xT_s[:], xT)
            lin1T_s = wp.tile([IN_DIM, D_MODEL], bf, tag="lin1T", name="lin1T")
            nc.gpsimd.dma_start(lin1T_s[:], lin1T)
            lin1b_s = wp.tile([P, KM], f32, tag="lin1b", name="lin1b")
            nc.gpsimd.dma_start(lin1b_s[:], lin1b)
            lin2Tp_s = wp.tile([P, KM], bf, tag="lin2Tp", name="lin2Tp")
            nc.gpsimd.dma_start(lin2Tp_s[:], lin2Tp)
            lin2b_s = wp.tile([1, 1], f32, tag="lin2b", name="lin2b")
            nc.gpsimd.dma_start(lin2b_s[:], lin2b)
            idn_s = wp.tile([P, P], bf, tag="idn", name="idn")
            nc.gpsimd.dma_start(idn_s[:], idn_d)

            w2q_s = wp.tile([P, JT], bf, tag="w2q", name="w2q")
            nc.gpsimd.dma_start(
                w2q_s[:], w2q_d.rearrange("(j p) one -> p (j one)", p=P))
            ones1 = wp.tile([1, P], bf, tag="ones1", name="ones1")      # K=1 bcast lhsT
            nc.vector.memset(ones1[:], 1.0)
            zconst = wp.tile([P, 1], f32, tag="zconst", name="zconst")
            nc.vector.memset(zconst[:], 0.0)
            nc.const_aps.aps[(dt.float32, 0.0)] = zconst
            epsconst = wp.tile([P, 1], f32, tag="epsconst", name="epsconst")
            nc.vector.memset(epsconst[:], EPS)
            nc.const_aps.aps[(dt.float32, EPS)] = epsconst
            oneconst = wp.tile([P, 1], f32, tag="oneconst", name="oneconst")
            nc.vector.memset(oneconst[:], 1.0)
            nc.const_aps.aps[(dt.float32, 1.0)] = oneconst
            onesk = wp.tile([P, 1], bf, tag="onesk", name="onesk")      # norm reduce lhsT
            nc.vector.memset(onesk[:], 1.0)
            ones64 = wp.tile([64, P], bf, tag="ones64", name="ones64")   # bcast lhsT at any base part
            nc.vector.memset(ones64[:], 1.0)

            ipx_s, ipz_s, convw_s, convb_s, xp_s, dtw_s, dtb_s, asc_s, dp_s, op_s = (
                [], [], [], [], [], [], [], [], [], [])
            for l in range(N_LAYERS):
                t_ = [wp.tile([P, D_INNER], bf, tag=f"ipx{l}_{k}", name=f"ipx{l}_{k}") for k in range(KM)]
                for k in range(KM):
                    nc.gpsimd.dma_start(t_[k][:], ipx_d[l][k * P:(k + 1) * P, :])
                ipx_s.append(t_)
                t_ = [wp.tile([P, QUART], bf, tag=f"ipz{l}_{k}", name=f"ipz{l}_{k}") for k in range(KM)]
                for k in range(KM):
                    nc.gpsimd.dma_start(t_[k][:], ipz_d[l][k * P:(k + 1) * P, :])
                ipz_s.append(t_)
                t_ = wp.tile([P, GX * D_CONV], f32, tag=f"convw{l}", name=f"convw{l}")
                nc.gpsimd.dma_start(t_[:], convw_d[l])
                convw_s.append(t_)
                t_ = wp.tile([P, GX], f32, tag=f"convb{l}", name=f"convb{l}")
                nc.gpsimd.dma_start(t_[:], convb_d[l])
                convb_s.append(t_)
                t_ = [wp.tile([P, DT_RANK + 2 * D_STATE], bf, tag=f"xp{l}_{k}", name=f"xp{l}_{k}") for k in range(GX)]
                for k in range(GX):
                    nc.gpsimd.dma_start(t_[k][:], xp_d[l][k * P:(k + 1) * P, :])
                xp_s.append(t_)
                t_ = wp.tile([DT_RANK, QUART], bf, tag=f"dtw{l}", name=f"dtw{l}")
                nc.gpsimd.dma_start(t_[:], dtw_d[l])
                dtw_s.append(t_)
                t_ = wp.tile([P, JT], f32, tag=f"dtb{l}", name=f"dtb{l}")
                nc.gpsimd.dma_start(t_[:], dtb_d[l])
                dtb_s.append(t_)
                t_ = wp.tile([P, JT * D_STATE], f32, tag=f"asc{l}", name=f"asc{l}")
                nc.gpsimd.dma_start(t_[:], asc_d[l])
                asc_s.append(t_)
                t_ = wp.tile([P, JT], f32, tag=f"dp{l}", name=f"dp{l}")
                nc.gpsimd.dma_start(t_[:], dp_d[l])
                dp_s.append(t_)
                t_ = [wp.tile([P, D_MODEL], bf, tag=f"op{l}_{k}", name=f"op{l}_{k}") for k in range(JT)]
                for k in range(JT):
                    nc.gpsimd.dma_start(t_[k][:], op_d[l][k * P:(k + 1) * P, :])
                op_s.append(t_)

            # ---------------- lin1: h = x @ lin1_w.T ----------------
            h = [hp.tile([P, T], bf, tag=f"h{m}", name=f"h{m}") for m in range(KM)]
            with tc.tile_pool(name="ps_lin1", bufs=2, space="PSUM") as pp:
                for m in range(KM):
                    for ch in range(NCH):
                        ps = pp.tile([P, 512], f32, tag="ps", name="ps")
                        nc.tensor.matmul(
                            ps[:], lin1T_s[:, m * P:(m + 1) * P],
                            xT_s[:, ch * 512:(ch + 1) * 512])
                        nc.scalar.activation(
                            h[m][:, ch * 512:(ch + 1) * 512], ps[:],
                            AF.Identity, bias=lin1b_s[:, m:m + 1])

            # ---------------- layers ----------------
            for rep in range(reps):
              for l in range(N_LAYERS):
                with tc.tile_pool(name=f"lay{rep}_{l}", bufs=1) as lp:
                    xin_c = [lp.tile([P, T], bf, tag=f"xinc{j}", name=f"xinc{j}")
                             for j in range(JT)]
                    sz = [lp.tile([P, T], bf, tag=f"sz{j}", name=f"sz{j}") for j in range(JT)]
                    dbc = lp.tile([DT_RANK + 2 * D_STATE, T], bf, tag="dbc", name="dbc")
                    delta = [lp.tile([P, T], bf, tag=f"delta{j}", name=f"delta{j}")
                             for j in range(JT)]
                    du = [lp.tile([P, T], bf, tag=f"du{j}", name=f"du{j}") for j in range(JT)]
                    du2 = [lp.tile([P, T], bf, tag=f"du2{j}", name=f"du2{j}") for j in range(JT)]
                    yg = [lp.tile([P, T], bf, tag=f"yg{j}", name=f"yg{j}") for j in range(JT)]

                    with (
                        tc.tile_pool(name="hnp", bufs=1) as hnp,
                        tc.tile_pool(name="ps_c", bufs=2, space="PSUM") as pp,
                        tc.tile_pool(name="xtra", bufs=3) as xtp,
                    ):
                        # ---- rmsnorm factor (norm_w folded into weights) ----
                        inv1 = hnp.tile([1, T], f32, tag="inv1", name="inv1")
                        sqs = [hnp.tile([P, T], bf, tag=f"sq{m}", name=f"sq{m}")
                               for m in range(KM)]
                        for m in range(KM):
                            nc.scalar.activation(sqs[m][:], h[m][:], AF.Square)
                        for ch in range(NCH):
                            ps1 = pp.tile([1, 512], f32, tag="ps", name="ps", bufs=3)
                            for m in range(KM):
                                nc.tensor.matmul(
                                    ps1[:], onesk[:],
                                    sqs[m][:, ch * 512:(ch + 1) * 512],
                                    start=(m == 0), stop=(m == KM - 1))
                            nc.scalar.activation(
                                inv1[:, ch * 512:(ch + 1) * 512], ps1[:],
                                AF.Ln, scale=1.0 / D_MODEL, bias=EPS)
                        inv1b = hnp.tile([1, T], bf, tag="inv1b", name="inv1b")
                        nc.scalar.activation(inv1b[:], inv1[:], AF.Exp, scale=-0.5)
                        invb = hnp.tile([P, T], bf, tag="invb", name="invb")
                        for ch in range(NCH):
                            psb = pp.tile([P, 512], f32, tag="ps", name="ps", bufs=3)
                            nc.tensor.matmul(
                                psb[:], ones1[:], inv1b[:, ch * 512:(ch + 1) * 512])
                            nc.scalar.activation(
                                invb[:, ch * 512:(ch + 1) * 512], psb[:], AF.Copy)

                        hn = [hnp.tile([P, T], bf, tag=f"hn{m}", name=f"hn{m}")
                              for m in range(KM)]
                        for m in range(KM):
                            nc.vector.tensor_tensor(hn[m][:], h[m][:], invb[:], OP.mult)

                        # ---- in_proj x-half (full D_INNER) + conv + silu + x_proj ----
                        xps = [pp.tile([DT_RANK + 2 * D_STATE, 512], f32,
                                       tag=f"xps{ch}", name=f"xps{ch}", bufs=1)
                               for ch in range(NCH)]
                        for g in range(GX):
                            xpad = xtp.tile([P, PAD + T], bf, tag="xpad", name="xpad")
                            nc.vector.memset(xpad[:, 0:PAD], 0.0)
                            for ch in range(NCH):
                                ps = pp.tile([P, 512], f32, tag="ps", name="ps", bufs=3)
                                for k in range(KM):
                                    nc.tensor.matmul(
                                        ps[:],
                                        ipx_s[l][k][:, g * P:(g + 1) * P],
                                        hn[k][:, ch * 512:(ch + 1) * 512],
                                        start=(k == 0), stop=(k == KM - 1))
                                nc.scalar.activation(
                                    xpad[:, PAD + ch * 512: PAD + (ch + 1) * 512],
                                    ps[:], AF.Copy)
                            # causal conv on DVE: 4x tensor_scalar taps + add tree
                            if g < JT:
                                xc = xin_c[g]
                            else:
                                xc = xtp.tile([P, T], bf, tag="xcrot", name="xcrot", bufs=3)
                            tp0 = xtp.tile([P, T], bf, tag="tp0", name="tp0", bufs=2)
                            tp1 = xtp.tile([P, T], bf, tag="tp1", name="tp1", bufs=2)
                            tp2 = xtp.tile([P, T], bf, tag="tp2", name="tp2", bufs=1)
                            tp3 = xtp.tile([P, T], bf, tag="tp3", name="tp3", bufs=1)
                            tps = [tp0, tp1, tp2, tp3]
                            for k in range(D_CONV):
                                nc.vector.tensor_scalar(
                                    tps[k][:], xpad[:, 1 + k:1 + k + T],
                                    convw_s[l][:, g * D_CONV + k:g * D_CONV + k + 1],
                                    None, OP.mult)
                            nc.vector.tensor_tensor(tp0[:], tp0[:], tp1[:], OP.add)
                            nc.vector.tensor_tensor(tp2[:], tp2[:], tp3[:], OP.add)
                            nc.vector.tensor_tensor(tp0[:], tp0[:], tp2[:], OP.add)
                            nc.scalar.activation(
                                xc[:], tp0[:], AF.Silu, bias=convb_s[l][:, g:g + 1])
                            for ch in range(NCH):
                                nc.tensor.matmul(
                                    xps[ch][:], xp_s[l][g][:],
                                    xc[:, ch * 512:(ch + 1) * 512],
                                    start=(g == 0), stop=(g == GX - 1))
                        # z-half (own quarter); silu directly out of psum
                        for j in range(JT):
                            for ch in range(NCH):
                                ps = pp.tile([P, 512], f32, tag="ps", name="ps", bufs=3)
                                for k in range(KM):
                                    nc.tensor.matmul(
                                        ps[:],
                                        ipz_s[l][k][:, j * P:(j + 1) * P],
                                        hn[k][:, ch * 512:(ch + 1) * 512],
                                        start=(k == 0), stop=(k == KM - 1))
                                nc.scalar.activation(
                                    sz[j][:, ch * 512:(ch + 1) * 512], ps[:], AF.Silu)
                        # evict dbc
                        for ch in range(NCH):
                            nc.scalar.activation(
                                dbc[:, ch * 512:(ch + 1) * 512], xps[ch][:], AF.Copy)
                        # ---- dt_proj -> delta (softplus), du ----
                        for j in range(JT):
                            for ch in range(NCH):
                                psd = pp.tile([P, 512], f32, tag="ps", name="ps", bufs=3)
                                nc.tensor.matmul(
                                    psd[:], dtw_s[l][:, j * P:(j + 1) * P],
                                    dbc[0:DT_RANK, ch * 512:(ch + 1) * 512])
                                # softplus(x) = ln(1 + exp(x)); same ACT
                                # table set as Exp/Ln used elsewhere
                                ex = xtp.tile([P, 512], f32, tag="ex", name="ex", bufs=2)
                                nc.scalar.activation(
                                    ex[:], psd[:], AF.Exp,
                                    bias=dtb_s[l][:, j:j + 1])
                                nc.scalar.activation(
                                    delta[j][:, ch * 512:(ch + 1) * 512], ex[:],
                                    AF.Ln, bias=1.0)
                            # own-quarter u tiles are xin_c[0..JT-1]
                            nc.vector.tensor_tensor(
                                du[j][:], delta[j][:], xin_c[j][:], OP.mult)
                            nc.vector.tensor_scalar(
                                du2[j][:], xin_c[j][:], dp_s[l][:, j:j + 1], None,
                                OP.mult)

                    # ---- scan stage ----
                    with (
                        tc.tile_pool(name="ps_y", bufs=1, space="PSUM") as pyp,
                        tc.tile_pool(name="scan", bufs=2) as scp,
                    ):
                        ypsum = [pyp.tile([P, T], f32, tag=f"ypsum{j}", name=f"ypsum{j}")
                                 for j in range(JT)]
                        for j in range(JT):
                            for ch in range(NCH):
                                nc.tensor.matmul(
                                    ypsum[j][:, ch * 512:(ch + 1) * 512],
                                    idn_s[:], du2[j][:, ch * 512:(ch + 1) * 512],
                                    start=True, stop=False)
                        for n in range(D_STATE):
                            Bb = scp.tile([P, T], bf, tag="Bb", name="Bb", bufs=6)
                            Cb = scp.tile([P, T], bf, tag="Cb", name="Cb", bufs=6)
                            nc.sync.dma_start(
                                Bb[0:1, :], dbc[DT_RANK + n:DT_RANK + n + 1, :])
                            nc.gpsimd.dma_start(
                                Cb[0:1, :], dbc[DT_RANK + D_STATE + n:
                                                DT_RANK + D_STATE + n + 1, :])
                            w = 1
                            while w < P:
                                nc.sync.dma_start(Bb[w:2 * w, :], Bb[0:w, :])
                                nc.gpsimd.dma_start(Cb[w:2 * w, :], Cb[0:w, :])
                                w *= 2
                            for j in range(JT):
                                dA = scp.tile([P, T], bf, tag="dA", name="dA")
                                nc.scalar.activation(
                                    dA[:], delta[j][:], AF.Exp,
                                    scale=asc_s[l][:, j * D_STATE + n:
                                                   j * D_STATE + n + 1])
                                bx = scp.tile([P, T], bf, tag="bx", name="bx")
                                nc.vector.tensor_tensor(bx[:], du[j][:], Bb[:], OP.mult)
                                hs = scp.tile([P, T], bf, tag="hs", name="hs")
                                nc.vector.tensor_tensor_scan(
                                    hs[:], dA[:], bx[:], 0.0, OP.mult, OP.add)
                                hc = scp.tile([P, T], bf, tag="hc", name="hc")
                                nc.vector.tensor_tensor(hc[:], hs[:], Cb[:], OP.mult)
                                for ch in range(NCH):
                                    nc.tensor.matmul(
                                        ypsum[j][:, ch * 512:(ch + 1) * 512],
                                        idn_s[:], hc[:, ch * 512:(ch + 1) * 512],
                                        start=False, stop=(n == D_STATE - 1))
                        # gate: yg = ypsum * silu(z)  (u*D already accumulated)
                        for j in range(JT):
                            t1 = scp.tile([P, T], bf, tag="t1", name="t1", bufs=2)
                            for ch in range(NCH):
                                nc.scalar.activation(
                                    t1[:, ch * 512:(ch + 1) * 512],
                                    ypsum[j][:, ch * 512:(ch + 1) * 512], AF.Copy)
                            nc.vector.tensor_tensor(yg[j][:], t1[:], sz[j][:], OP.mult)

                    if l == N_LAYERS - 1 and rep == reps - 1:
                        # ---- folded: r = (lin2_w @ op_w[:,shard]) @ yg; tiny AR ----
                        with (
                            tc.tile_pool(name="ps_r", bufs=2, space="PSUM") as pp,
                            tc.tile_pool(name="rp", bufs=1) as rpp,
                        ):
                            ar2_in = dramp.tile([1, T], f32, tag="ar2in", name="ar2in")
                            ar2_out = dramp.tile([1, T], f32, tag="ar2out", name="ar2out")
                            rp = rpp.tile([1, T], f32, tag="rp", name="rp")
                            for ch in range(NCH):
                                ps = pp.tile([1, 512], f32, tag="ps", name="ps")
                                for j in range(JT):
                                    nc.tensor.matmul(
                                        ps[:], w2q_s[:, j:j + 1],
                                        yg[j][:, ch * 512:(ch + 1) * 512],
                                        start=(j == 0), stop=(j == JT - 1))
                                nc.scalar.activation(
                                    rp[:, ch * 512:(ch + 1) * 512], ps[:], AF.Copy)
                            nc.sync.dma_start(ar2_in[:], rp[:])
                            nc.gpsimd.collective_compute(
                                "AllReduce", OP.add,
                                replica_groups=[[0, 1, 2, 3], [4, 5, 6, 7]],
                                ins=[ar2_in.opt()], outs=[ar2_out.opt()])
                            # lin2 on pre-residual h (overlaps layer-1 compute)
                            l2h = rpp.tile([1, T], f32, tag="l2h", name="l2h")
                            with tc.tile_pool(name="ps_l2b", bufs=2, space="PSUM") as p2:
                                for ch in range(NCH):
                                    ps = p2.tile([1, 512], f32, tag="ps", name="ps")
                                    for k in range(KM):
                                        nc.tensor.matmul(
                                            ps[:], lin2Tp_s[:, k:k + 1],
                                            h[k][:, ch * 512:(ch + 1) * 512],
                                            start=(k == 0), stop=(k == KM - 1))
                                    nc.scalar.activation(
                                        l2h[:, ch * 512:(ch + 1) * 512], ps[:], AF.Copy)
                            arsb = rpp.tile([1, T], f32, tag="arsb", name="arsb")
                            nc.sync.dma_start(arsb[:], ar2_out[:])
                            ysum = rpp.tile([1, T], f32, tag="ysum", name="ysum")
                            nc.vector.tensor_tensor(ysum[:], l2h[:], arsb[:], OP.add)
                            yrow = rpp.tile([1, T], f32, tag="yrow", name="yrow")
                            nc.scalar.activation(
                                yrow[:], ysum[:], AF.Sigmoid, bias=lin2b_s[:])
                            nc.sync.dma_start(yrow_d, yrow[:])
                        continue

                    # ---- out_proj partial + AllReduce + residual ----
                    with (
                        tc.tile_pool(name="ps_op", bufs=2, space="PSUM") as pp,
                        tc.tile_pool(name="arp", bufs=1) as arp,
                    ):
                        ar_in = dramp.tile([D_MODEL, T], bf, tag=f"arin{rep}_{l}",
                                           name=f"arin{rep}_{l}")
                        ar_out = dramp.tile([D_MODEL, T], bf, tag=f"arout{rep}_{l}",
                                            name=f"arout{rep}_{l}")
                        part = arp.tile([P, KM * T], bf, tag="part", name="part")
                        for m in range(KM):
                            for ch in range(NCH):
                                ps = pp.tile([P, 512], f32, tag="ps", name="ps")
                                for j in range(JT):
                                    nc.tensor.matmul(
                                        ps[:], op_s[l][j][:, m * P:(m + 1) * P],
                                        yg[j][:, ch * 512:(ch + 1) * 512],
                                        start=(j == 0), stop=(j == JT - 1))
                                nc.scalar.activation(
                                    part[:, m * T + ch * 512:
                                         m * T + (ch + 1) * 512], ps[:], AF.Copy)
                            (nc.sync if m % 2 == 0 else nc.gpsimd).dma_start(
                                ar_in[m * P:(m + 1) * P, :],
                                part[:, m * T:(m + 1) * T])
                        if use_cc:
                            nc.gpsimd.collective_compute(
                                "AllReduce", OP.add,
                                replica_groups=[[0, 1, 2, 3], [4, 5, 6, 7]],
                                ins=[ar_in.opt()], outs=[ar_out.opt()])
                        else:
                            ar_out = ar_in
                        for m in range(KM):
                            hd = arp.tile([P, T], bf, tag="hd", name="hd", bufs=2)
                            (nc.sync if m % 2 == 0 else nc.gpsimd).dma_start(
                                hd[:], ar_out[m * P:(m + 1) * P, :])
                            nc.vector.tensor_tensor(h[m][:], h[m][:], hd[:], OP.add)

    nc.compile()
    _CACHE[key] = nc
    return nc


def _prep_inputs(inputs):
    """Host-side prep: per-core input maps.

    Own-quarter reordering: so that the device program is uniform across
    cores, each core's xin tiles are ordered with its OWN quarter first
    (tiles 0..1), then the remaining quarters in cyclic order.  All tensors
    indexed by d_inner on the x-path (ipx columns, conv w/b, xp rows) are
    permuted consistently on the host, so dbc/x_proj results are unchanged.
    """
    f32 = np.float32
    x = np.asarray(inputs["x"], f32)
    lin1_w = np.asarray(inputs["lin1_w"], f32)
    lin1_b = np.asarray(inputs["lin1_b"], f32)
    lin2_w = np.asarray(inputs["lin2_w"], f32)
    lin2_b = np.asarray(inputs["lin2_b"], f32)
    norm_w = np.asarray(inputs["norm_w"], f32)
    in_proj_w = np.asarray(inputs["in_proj_w"], f32)
    conv_w = np.asarray(inputs["conv_w"], f32)
    conv_b = np.asarray(inputs["conv_b"], f32)
    x_proj_w = np.asarray(inputs["x_proj_w"], f32)
    dt_proj_w = np.asarray(inputs["dt_proj_w"], f32)
    dt_proj_b = np.asarray(inputs["dt_proj_b"], f32)
    A_log = np.asarray(inputs["A_log"], f32)
    D_param = np.asarray(inputs["D_param"], f32)
    out_proj_w = np.asarray(inputs["out_proj_w"], f32)

    A = -np.exp(A_log)  # (N_LAYERS, D_INNER, D_STATE)
    bf = np.dtype("bfloat16") if hasattr(np, "bfloat16") else None
    import ml_dtypes
    bf = ml_dtypes.bfloat16

    def b16(a):
        return np.ascontiguousarray(a).astype(bf)

    in_maps = []
    for c in range(N_CORES):
        b = c // 4
        q = c % 4
        # cyclic quarter order: own quarter first
        qorder = [(q + i) % 4 for i in range(4)]
        ch_perm = np.concatenate([
            np.arange(qq * QUART, (qq + 1) * QUART) for qq in qorder])

        m = {}
        m["xT"] = b16(x[b].T)                                   # [16, T]
        m["lin1T"] = b16(lin1_w.T)                              # [16, 512]
        m["lin1b"] = np.ascontiguousarray(
            lin1_b.reshape(KM, P).T).astype(f32)                # [128, 4]
        m["lin2Tp"] = b16(lin2_w[0].reshape(KM, P).T)           # [128, 4]
        m["lin2b"] = lin2_b.reshape(1, 1).astype(f32)
        m["idn"] = b16(np.eye(P))

        for l in range(N_LAYERS):
            wn = in_proj_w[l] * norm_w[l][None, :]              # [2048, 512]
            ipx = wn[:D_INNER][ch_perm]                         # perm rows
            m[f"ipx{l}"] = b16(ipx.T)                           # [512, 1024]
            zsh = wn[D_INNER + q * QUART: D_INNER + (q + 1) * QUART]
            m[f"ipz{l}"] = b16(zsh.T)                           # [512, 256]

            cw = conv_w[l, :, 0, :][ch_perm]                    # [1024, 4]
            m[f"convw{l}"] = np.ascontiguousarray(
                cw.reshape(GX, P, D_CONV).transpose(1, 0, 2).reshape(P, GX * D_CONV)
            ).astype(f32)
            cb = conv_b[l][ch_perm]
            m[f"convb{l}"] = np.ascontiguousarray(
                cb.reshape(GX, P).T).astype(f32)

            m[f"xp{l}"] = b16(x_proj_w[l].T[ch_perm])           # [1024, 64]

            sh = slice(q * QUART, (q + 1) * QUART)
            m[f"dtw{l}"] = b16(dt_proj_w[l, sh].T)              # [32, 256]
            m[f"dtb{l}"] = np.ascontiguousarray(
                dt_proj_b[l, sh].reshape(JT, P).T).astype(f32)  # [128, 2]
            m[f"asc{l}"] = np.ascontiguousarray(
                A[l, sh].reshape(JT, P, D_STATE).transpose(1, 0, 2)
                .reshape(P, JT * D_STATE)).astype(f32)          # [128, 32]
            m[f"dp{l}"] = np.ascontiguousarray(
                D_param[l, sh].reshape(JT, P).T).astype(f32)    # [128, 2]
            m[f"op{l}"] = b16(out_proj_w[l][:, sh].T)           # [256, 512]
        m["w2q"] = b16((lin2_w[0:1, :] @ out_proj_w[N_LAYERS - 1][:, q * QUART:(q + 1) * QUART]).T)
        in_maps.append(m)
    return in_maps


def kernel(**inputs):
    nc = _build_program()
    in_maps = _prep_inputs(inputs)
    res = run_bass_kernel_spmd(nc, in_maps, core_ids=list(range(N_CORES)))
    out = np.zeros((B, L), np.float32)
    for b in range(B):
        out[b] = res.results[b * 4]["yrow"][0]
    return out


if __name__ == "__main__":
    import reference
    inp = reference.setup_inputs()
    exp = np.asarray(reference.reference(**inp))
    act = kernel(**{k: np.asarray(v) for k, v in inp.items()})
    err = np.abs(act - exp).max() / (np.abs(exp).max() + 1e-12)
    print("max abs err:", np.abs(act - exp).max(), "rel:", err)



# revision 3
# speedup vs baseline: 1.0817x; 1.0817x over previous
"""Mamba-2-layer net on 8 trn2 NeuronCores — v2.

Sharding: core c -> batch b = c // 4, d_inner quarter q = c % 4 (256 ch).
v2 changes vs baseline:
  - TP-split x-path: each core computes in_proj-x/conv/x_proj only for its
    own quarter; a small f32 AllReduce produces dbc (was: 4x-replicated).
  - Time-halved pipeline (HT=1024): out_proj -> AllReduce -> residual ->
    next-layer phase A run per half and are emitted *inside* the other
    half's scan loop so PE/ACT/collective work hides under the DVE scans.
  - Scans chained across halves via per-(n,j) carry columns.
  - Silu ops batched to limit ACT table loads.
"""

import sys
import numpy as np

sys.path.insert(0, "/opt/trn_rl_repo")

import concourse.bass as bass
import concourse.bacc as bacc
import concourse.tile as tile
import concourse.mybir as mybir
from concourse.bass_utils import run_bass_kernel_spmd

dt = mybir.dt
AF = mybir.ActivationFunctionType
OP = mybir.AluOpType

# model dims
B, L = 2, 2048
IN_DIM = 16
D_MODEL = 512
D_INNER = 1024
D_STATE = 16
D_CONV = 4
DT_RANK = 32
N_LAYERS = 2
EPS = 1e-5

# sharding
N_CORES = 8
QUART = D_INNER // 4      # 256 channels per core
T = L
HT = T // 2               # half-T pipeline granularity
P = 128
JT = QUART // P           # 2 d-tiles per quarter
KM = D_MODEL // P         # 4 k-tiles over d_model
PAD = 4
RG = [[0, 1, 2, 3], [4, 5, 6, 7]]

_CACHE = {}


def _build_program(reps=1, use_cc=True, single_core=False):
    key = ("prog2", reps, use_cc, single_core)
    if key in _CACHE:
        return _CACHE[key]

    nc = bacc.Bacc(
        "TRN2",
        target_bir_lowering=False,
        debug=False,
        enable_asserts=False,
        num_devices=1 if single_core else N_CORES,
    )

    bf = dt.bfloat16
    f32 = dt.float32

    # ---------------- DRAM I/O ----------------
    xT = nc.dram_tensor("xT", [IN_DIM, T], bf, kind="ExternalInput").ap()
    lin1T = nc.dram_tensor("lin1T", [IN_DIM, D_MODEL], bf, kind="ExternalInput").ap()
    lin1b = nc.dram_tensor("lin1b", [P, KM], f32, kind="ExternalInput").ap()
    lin2Tp = nc.dram_tensor("lin2Tp", [P, KM], bf, kind="ExternalInput").ap()
    lin2b = nc.dram_tensor("lin2b", [1, 1], f32, kind="ExternalInput").ap()
    idn_d = nc.dram_tensor("idn", [P, P], bf, kind="ExternalInput").ap()

    ipxq_d, ipz_d, convw_d, convb_d, xpq_d, dtw_d, dtb_d, asc_d, dp_d, op_d = (
        [], [], [], [], [], [], [], [], [], [])
    for l in range(N_LAYERS):
        ipxq_d.append(nc.dram_tensor(f"ipxq{l}", [D_MODEL, QUART], bf, kind="ExternalInput").ap())
        ipz_d.append(nc.dram_tensor(f"ipz{l}", [D_MODEL, QUART], bf, kind="ExternalInput").ap())
        convw_d.append(nc.dram_tensor(f"convw{l}", [P, JT * D_CONV], f32, kind="ExternalInput").ap())
        convb_d.append(nc.dram_tensor(f"convb{l}", [P, JT], f32, kind="ExternalInput").ap())
        xpq_d.append(nc.dram_tensor(f"xpq{l}", [QUART, DT_RANK + 2 * D_STATE], bf, kind="ExternalInput").ap())
        dtw_d.append(nc.dram_tensor(f"dtw{l}", [DT_RANK, QUART], bf, kind="ExternalInput").ap())
        dtb_d.append(nc.dram_tensor(f"dtb{l}", [P, JT], f32, kind="ExternalInput").ap())
        asc_d.append(nc.dram_tensor(f"asc{l}", [P, JT * D_STATE], f32, kind="ExternalInput").ap())
        dp_d.append(nc.dram_tensor(f"diagD{l}", [P, JT * P], bf, kind="ExternalInput").ap())
        op_d.append(nc.dram_tensor(f"op{l}", [QUART, D_MODEL], bf, kind="ExternalInput").ap())

    w2q_d = nc.dram_tensor("w2q", [QUART, 1], bf, kind="ExternalInput").ap()
    yrow_d = nc.dram_tensor("yrow", [1, T], f32, kind="ExternalOutput").ap()

    NDBC = DT_RANK + 2 * D_STATE  # 64

    with tile.TileContext(nc) as tc:
        with (
            tc.tile_pool(name="wp", bufs=1) as wp,
            tc.tile_pool(name="tp", bufs=1) as tp,
            tc.tile_pool(name="scp", bufs=1) as scp,
            tc.tile_pool(name="bcp", bufs=1) as bcp,
            tc.tile_pool(name="pp", bufs=1, space="PSUM") as pp,
            tc.tile_pool(name="dramp", bufs=1, space="DRAM") as dramp,
        ):
            # ---------------- weight loads ----------------
            xT_s = wp.tile([IN_DIM, T], bf, tag="xT", name="xT")
            nc.sync.dma_start(xT_s[:], xT)
            lin1T_s = wp.tile([IN_DIM, D_MODEL], bf, tag="lin1T", name="lin1T")
            nc.sync.dma_start(lin1T_s[:], lin1T)
            lin1b_s = wp.tile([P, KM], f32, tag="lin1b", name="lin1b")
            nc.sync.dma_start(lin1b_s[:], lin1b)
            idn_s = wp.tile([P, P], bf, tag="idn", name="idn")
            nc.sync.dma_start(idn_s[:], idn_d)
            lin2Tp_s = wp.tile([P, KM], bf, tag="lin2Tp", name="lin2Tp")
            nc.gpsimd.dma_start(lin2Tp_s[:], lin2Tp)
            lin2b_s = wp.tile([1, 1], f32, tag="lin2b", name="lin2b")
            nc.gpsimd.dma_start(lin2b_s[:], lin2b)
            w2q_s = wp.tile([P, JT], bf, tag="w2q", name="w2q")
            nc.gpsimd.dma_start(
                w2q_s[:], w2q_d.rearrange("(j p) one -> p (j one)", p=P))

            ones1 = wp.tile([1, P], bf, tag="ones1", name="ones1")
            nc.vector.memset(ones1[:], 1.0)
            onesk = wp.tile([P, 1], bf, tag="onesk", name="onesk")
            nc.vector.memset(onesk[:], 1.0)
            zconst = wp.tile([P, 1], f32, tag="zconst", name="zconst")
            nc.vector.memset(zconst[:], 0.0)
            nc.const_aps.aps[(dt.float32, 0.0)] = zconst
            epsconst = wp.tile([P, 1], f32, tag="epsconst", name="epsconst")
            nc.vector.memset(epsconst[:], EPS)
            nc.const_aps.aps[(dt.float32, EPS)] = epsconst
            oneconst = wp.tile([P, 1], f32, tag="oneconst", name="oneconst")
            nc.vector.memset(oneconst[:], 1.0)
            nc.const_aps.aps[(dt.float32, 1.0)] = oneconst

            ipxq_s, ipz_s, convw_s, convb_s, xpq_s, dtw_s, dtb_s, asc_s, dp_s, op_s = (
                [], [], [], [], [], [], [], [], [], [])
            for l in range(N_LAYERS):
                q0 = nc.sync if l == 0 else nc.gpsimd
                t_ = [wp.tile([P, QUART], bf, tag=f"ipxq{l}_{k}", name=f"ipxq{l}_{k}") for k in range(KM)]
                for k in range(KM):
                    q0.dma_start(t_[k][:], ipxq_d[l][k * P:(k + 1) * P, :])
                ipxq_s.append(t_)
                t_ = [wp.tile([P, QUART], bf, tag=f"ipz{l}_{k}", name=f"ipz{l}_{k}") for k in range(KM)]
                for k in range(KM):
                    q0.dma_start(t_[k][:], ipz_d[l][k * P:(k + 1) * P, :])
                ipz_s.append(t_)
                t_ = wp.tile([P, JT * D_CONV], f32, tag=f"convw{l}", name=f"convw{l}")
                q0.dma_start(t_[:], convw_d[l])
                convw_s.append(t_)
                t_ = wp.tile([P, JT], f32, tag=f"convb{l}", name=f"convb{l}")
                q0.dma_start(t_[:], convb_d[l])
                convb_s.append(t_)
                t_ = [wp.tile([P, NDBC], bf, tag=f"xpq{l}_{g}", name=f"xpq{l}_{g}") for g in range(JT)]
                for g in range(JT):
                    q0.dma_start(t_[g][:], xpq_d[l][g * P:(g + 1) * P, :])
                xpq_s.append(t_)
                t_ = wp.tile([DT_RANK, QUART], bf, tag=f"dtw{l}", name=f"dtw{l}")
                q0.dma_start(t_[:], dtw_d[l])
                dtw_s.append(t_)
                t_ = wp.tile([P, JT], f32, tag=f"dtb{l}", name=f"dtb{l}")
                q0.dma_start(t_[:], dtb_d[l])
                dtb_s.append(t_)
                t_ = wp.tile([P, JT * D_STATE], f32, tag=f"asc{l}", name=f"asc{l}")
                q0.dma_start(t_[:], asc_d[l])
                asc_s.append(t_)
                t_ = wp.tile([P, JT * P], bf, tag=f"diagD{l}", name=f"diagD{l}")
                q0.dma_start(t_[:], dp_d[l])
                dp_s.append(t_)
                t_ = [wp.tile([P, D_MODEL], bf, tag=f"op{l}_{j}", name=f"op{l}_{j}") for j in range(JT)]
                for j in range(JT):
                    nc.gpsimd.dma_start(t_[j][:], op_d[l][j * P:(j + 1) * P, :])
                op_s.append(t_)

            # ---------------- persistent tiles ----------------
            h_t = [wp.tile([P, T], bf, tag=f"h{m}", name=f"h{m}") for m in range(KM)]
            hn = [wp.tile([P, T], bf, tag=f"hn{m}", name=f"hn{m}") for m in range(KM)]
            xpad = [[wp.tile([P, PAD + T], bf, tag=f"xpad{l}_{g}", name=f"xpad{l}_{g}")
                     for g in range(JT)] for l in range(N_LAYERS)]
            carry = [wp.tile([P, D_STATE], bf, tag=f"carry{j}", name=f"carry{j}")
                     for j in range(JT)]
            rp = wp.tile([1, T], bf, tag="rp", name="rp")
            l2h = wp.tile([1, T], bf, tag="l2h", name="l2h")

            # DRAM AR buffers
            dbc_ar = [[(dramp.tile([NDBC, HT], bf, tag=f"dbci{l}_{h}", name=f"dbci{l}_{h}"),
                        dramp.tile([NDBC, HT], bf, tag=f"dbco{l}_{h}", name=f"dbco{l}_{h}"))
                       for h in range(2)] for l in range(N_LAYERS)]
            h_ar = [(dramp.tile([D_MODEL, HT], bf, tag=f"ari{h}", name=f"ari{h}"),
                     dramp.tile([D_MODEL, HT], bf, tag=f"aro{h}", name=f"aro{h}"))
                    for h in range(2)]
            ar2_in = dramp.tile([1, T], bf, tag="ar2in", name="ar2in")
            ar2_out = dramp.tile([1, T], bf, tag="ar2out", name="ar2out")

            # per-(l,h) context (transient tile handles)
            ctx = {}

            # ---------------- lin1 ----------------
            for ch in range(4):
                for m in range(KM):
                    ps = pp.tile([P, 512], f32, tag="ps", name="ps", bufs=2)
                    nc.tensor.matmul(
                        ps[:], lin1T_s[:, m * P:(m + 1) * P],
                        xT_s[:, ch * 512:(ch + 1) * 512])
                    nc.scalar.activation(
                        h_t[m][:, ch * 512:(ch + 1) * 512], ps[:],
                        AF.Identity, bias=lin1b_s[:, m:m + 1])

            # ---------------- emitters ----------------
            def emit_rms_act(l, h):
                """rmsnorm ACT/PE chain -> invb (no DVE)."""
                hsl = slice(h * HT, (h + 1) * HT)
                ps1 = [pp.tile([P, 512], f32, tag="ps", name="ps", bufs=2)
                       for ch in range(2)]
                for m in range(KM):
                    sq = tp.tile([P, HT], bf, tag="sq", name="sq", bufs=1)
                    nc.scalar.activation(sq[:], h_t[m][:, hsl], AF.Square)
                    for ch in range(2):
                        nc.tensor.matmul(
                            ps1[ch][0:1, :], onesk[:],
                            sq[:, ch * 512:(ch + 1) * 512],
                            start=(m == 0), stop=(m == KM - 1))
                inv1 = tp.tile([1, HT], f32, tag="inv1", name="inv1", bufs=1)
                for ch in range(2):
                    nc.scalar.activation(
                        inv1[:, ch * 512:(ch + 1) * 512], ps1[ch][0:1, :],
                        AF.Ln, scale=1.0 / D_MODEL, bias=EPS)
                inv1b = tp.tile([1, HT], bf, tag="inv1b", name="inv1b", bufs=1)
                nc.scalar.activation(inv1b[:], inv1[:], AF.Exp, scale=-0.5)
                invb = tp.tile([P, HT], bf, tag="invb", name="invb", bufs=1)
                for ch in range(2):
                    psb = pp.tile([P, 512], f32, tag="ps", name="ps", bufs=2)
                    nc.tensor.matmul(
                        psb[:], ones1[:], inv1b[:, ch * 512:(ch + 1) * 512])
                    nc.scalar.activation(
                        invb[:, ch * 512:(ch + 1) * 512], psb[:], AF.Copy)
                ctx[(l, h)] = {"invb": invb}

            def emit_rms_hn(l, h):
                """hn = h * invb (DVE)."""
                hsl = slice(h * HT, (h + 1) * HT)
                invb = ctx[(l, h)]["invb"]
                for m in range(KM):
                    nc.vector.tensor_tensor(
                        hn[m][:, hsl], h_t[m][:, hsl], invb[:], OP.mult)

            def emit_xz_mm(l, h):
                """in_proj x-half matmuls (own quarter) -> xpad (PE+ACT)."""
                if h == 0:
                    for g in range(JT):
                        nc.vector.memset(xpad[l][g][:, 0:PAD], 0.0)
                for g in range(JT):
                    for ch in range(2):
                        ps = pp.tile([P, 512], f32, tag="ps", name="ps", bufs=2)
                        for k in range(KM):
                            nc.tensor.matmul(
                                ps[:],
                                ipxq_s[l][k][:, g * P:(g + 1) * P],
                                hn[k][:, h * HT + ch * 512:h * HT + (ch + 1) * 512],
                                start=(k == 0), stop=(k == KM - 1))
                        nc.scalar.activation(
                            xpad[l][g][:, PAD + h * HT + ch * 512:
                                       PAD + h * HT + (ch + 1) * 512],
                            ps[:], AF.Copy)

            def emit_conv(l, h):
                """depthwise conv taps + adds (DVE only)."""
                conv_acc = []
                for g in range(JT):
                    tp0 = tp.tile([P, HT], bf, tag=f"tp0_{g}", name=f"tp0_{g}", bufs=1)
                    tp1 = tp.tile([P, HT], bf, tag="tp1", name="tp1", bufs=1)
                    tp2 = tp.tile([P, HT], bf, tag="tp2", name="tp2", bufs=1)
                    tp3 = tp.tile([P, HT], bf, tag="tp3", name="tp3", bufs=1)
                    tps = [tp0, tp1, tp2, tp3]
                    for k in range(D_CONV):
                        nc.vector.tensor_scalar(
                            tps[k][:], xpad[l][g][:, 1 + k + h * HT:1 + k + h * HT + HT],
                            convw_s[l][:, g * D_CONV + k:g * D_CONV + k + 1],
                            None, OP.mult)
                    nc.vector.tensor_tensor(tp0[:], tp0[:], tp1[:], OP.add)
                    nc.vector.tensor_tensor(tp2[:], tp2[:], tp3[:], OP.add)
                    nc.vector.tensor_tensor(tp0[:], tp0[:], tp2[:], OP.add)
                    conv_acc.append(tp0)
                ctx[(l, h)]["conv_acc"] = conv_acc

            def emit_xz_fin(l, h):
                """batched silus (conv + z), x_proj partial, dbc AR send."""
                c = ctx[(l, h)]
                xin = [tp.tile([P, HT], bf, tag=f"xin{g}", name=f"xin{g}", bufs=2)
                       for g in range(JT)]
                sz = [tp.tile([P, HT], bf, tag=f"sz{j}", name=f"sz{j}", bufs=2)
                      for j in range(JT)]
                for g in range(JT):
                    nc.scalar.activation(
                        xin[g][:], c["conv_acc"][g][:], AF.Silu,
                        bias=convb_s[l][:, g:g + 1])
                # x_proj partial (contract own 256 channels)
                xps = pp.tile([NDBC, HT], f32, tag="xps", name="xps", bufs=1)
                for g in range(JT):
                    for ch in range(2):
                        nc.tensor.matmul(
                            xps[:, ch * 512:(ch + 1) * 512], xpq_s[l][g][:],
                            xin[g][:, ch * 512:(ch + 1) * 512],
                            start=(g == 0), stop=(g == JT - 1))
                # evict + AR send (before z so the collective fires early)
                dbc_p = tp.tile([NDBC, HT], bf, tag="dbcp", name="dbcp", bufs=1)
                for ch in range(2):
                    nc.scalar.activation(
                        dbc_p[:, ch * 512:(ch + 1) * 512],
                        xps[:, ch * 512:(ch + 1) * 512], AF.Copy)
                ar_i, ar_o = dbc_ar[l][h]
                nc.sync.dma_start(ar_i[:], dbc_p[:])
                if use_cc:
                    nc.gpsimd.collective_compute(
                        "AllReduce", OP.add, replica_groups=RG,
                        ins=[ar_i.opt()], outs=[ar_o.opt()])
                # z-path: mm -> silu per chunk through the ps ring
                for j in range(JT):
                    for ch in range(2):
                        ps = pp.tile([P, 512], f32, tag="ps", name="ps", bufs=2)
                        for k in range(KM):
                            nc.tensor.matmul(
                                ps[:],
                                ipz_s[l][k][:, j * P:(j + 1) * P],
                                hn[k][:, h * HT + ch * 512:h * HT + (ch + 1) * 512],
                                start=(k == 0), stop=(k == KM - 1))
                        nc.scalar.activation(
                            sz[j][:, ch * 512:(ch + 1) * 512], ps[:], AF.Copy)
                c.update({"xin": xin, "sz": sz})

            def emit_delta(l, h):
                """dbc recv, dt_proj, softplus, du, du2."""
                c = ctx[(l, h)]
                ar_i, ar_o = dbc_ar[l][h]
                dbc = tp.tile([DT_RANK, HT], bf, tag="dbc", name="dbc", bufs=2)
                src_ar = (ar_o if use_cc else ar_i)
                nc.sync.dma_start(dbc[:], src_ar[0:DT_RANK, :])
                dbc4 = tp.tile([P, HT], bf, tag="dbc4", name="dbc4", bufs=2)
                for a in range(4):
                    (nc.sync if a % 2 == 0 else nc.scalar).dma_start(
                        dbc4[32 * a:32 * (a + 1), :], src_ar[DT_RANK:NDBC, :])
                delta = [tp.tile([P, HT], bf, tag=f"delta{j}", name=f"delta{j}", bufs=2)
                         for j in range(JT)]
                du = [tp.tile([P, HT], bf, tag=f"du{j}", name=f"du{j}", bufs=2)
                      for j in range(JT)]
                ex = tp.tile([P, 4 * 512], bf, tag="ex", name="ex", bufs=1)
                for j in range(JT):
                    for ch in range(2):
                        psd = pp.tile([P, 512], f32, tag="ps", name="ps", bufs=2)
                        nc.tensor.matmul(
                            psd[:], dtw_s[l][:, j * P:(j + 1) * P],
                            dbc[:, ch * 512:(ch + 1) * 512])
                        nc.scalar.activation(
                            ex[:, (j * 2 + ch) * 512:(j * 2 + ch + 1) * 512],
                            psd[:], AF.Exp, bias=dtb_s[l][:, j:j + 1])
                for j in range(JT):
                    for ch in range(2):
                        nc.scalar.activation(
                            delta[j][:, ch * 512:(ch + 1) * 512],
                            ex[:, (j * 2 + ch) * 512:(j * 2 + ch + 1) * 512],
                            AF.Ln, bias=1.0)
                for j in range(JT):
                    nc.vector.tensor_tensor(
                        du[j][:], delta[j][:], c["xin"][j][:], OP.mult)
                c.update({"dbc": dbc, "dbc4": dbc4, "delta": delta, "du": du})

            def emit_bcast(l, h, n):
                """B/C row broadcast via doubling DMAs for state n."""
                c = ctx[(l, h)]
                Bb = bcp.tile([P, HT], bf, tag="Bb", name="Bb", bufs=4)
                Cb = bcp.tile([P, HT], bf, tag="Cb", name="Cb", bufs=4)
                cq = nc.gpsimd if n % 2 == 0 else nc.scalar
                d4 = c["dbc4"]
                nc.sync.dma_start(Bb[0:4, :], d4[n:n + 97:32, :])
                cq.dma_start(Cb[0:4, :], d4[D_STATE + n:D_STATE + n + 97:32, :])
                w = 4
                while w < P:
                    nc.sync.dma_start(Bb[w:2 * w, :], Bb[0:w, :])
                    cq.dma_start(Cb[w:2 * w, :], Cb[0:w, :])
                    w *= 2
                return Bb, Cb

            def emit_scans(l, h, inject=None, pre=None):
                """16-state scan loop for half h; inject: {n: [fns]} emitted
                at state n (work that hides under the scans); pre: {n: (Bb, Cb)}
                broadcasts already emitted."""
                inject = inject or {}
                pre = pre or {}
                c = ctx[(l, h)]
                ypsum = [pp.tile([P, HT], f32, tag=f"ypsum{j}", name=f"ypsum{j}", bufs=1)
                         for j in range(JT)]
                c["ypsum"] = ypsum
                for j in range(JT):
                    for ch in range(2):
                        nc.tensor.matmul(
                            ypsum[j][:, ch * 512:(ch + 1) * 512],
                            dp_s[l][:, j * P:(j + 1) * P],
                            c["xin"][j][:, ch * 512:(ch + 1) * 512],
                            start=True, stop=False)
                for n in range(D_STATE):
                    for fn in inject.get(n, []):
                        fn()
                    if n in pre:
                        Bb, Cb = pre[n]
                    else:
                        Bb, Cb = emit_bcast(l, h, n)
                    Bb = Bb[:]
                    Cb = Cb[:]
                    for j in range(JT):
                        dA = scp.tile([P, HT], bf, tag="dA", name="dA", bufs=2)
                        nc.scalar.activation(
                            dA[:], c["delta"][j][:], AF.Exp,
                            scale=asc_s[l][:, j * D_STATE + n:j * D_STATE + n + 1])
                        bx = scp.tile([P, HT], bf, tag="bx", name="bx", bufs=2)
                        nc.vector.tensor_tensor(bx[:], c["du"][j][:], Bb, OP.mult)
                        hs = scp.tile([P, HT], bf, tag="hs", name="hs", bufs=2)
                        init = 0.0 if h == 0 else carry[j][:, n:n + 1]
                        nc.vector.tensor_tensor_scan(
                            hs[:], dA[:], bx[:], init, OP.mult, OP.add)
                        if h == 0:
                            nc.vector.tensor_copy(
                                carry[j][:, n:n + 1], hs[:, HT - 1:HT])
                        hc = scp.tile([P, HT], bf, tag="hc", name="hc", bufs=2)
                        nc.vector.tensor_tensor(hc[:], hs[:], Cb, OP.mult)
                        for ch in range(2):
                            nc.tensor.matmul(
                                ypsum[j][:, ch * 512:(ch + 1) * 512],
                                idn_s[:], hc[:, ch * 512:(ch + 1) * 512],
                                start=False, stop=(n == D_STATE - 1))

            def emit_yg(l, h):
                """ypsum -> gated yg (evicts ypsum psum)."""
                c = ctx[(l, h)]
                yg = [tp.tile([P, HT], bf, tag=f"yg{j}", name=f"yg{j}", bufs=1)
                      for j in range(JT)]
                t1s = []
                for j in range(JT):
                    t1 = tp.tile([P, HT], bf, tag=f"t1_{j}", name=f"t1_{j}", bufs=1)
                    for ch in range(2):
                        nc.scalar.activation(
                            t1[:, ch * 512:(ch + 1) * 512],
                            c["ypsum"][j][:, ch * 512:(ch + 1) * 512], AF.Copy)
                    t1s.append(t1)
                for j in range(JT):
                    nc.scalar.activation(
                        c["sz"][j][:], c["sz"][j][:], AF.Silu)
                for j in range(JT):
                    nc.vector.tensor_tensor(yg[j][:], t1s[j][:], c["sz"][j][:], OP.mult)
                c["yg"] = yg

            def emit_outproj(l, h):
                """out_proj partial + AR send (layer 0 only)."""
                c = ctx[(l, h)]
                ar_i, ar_o = h_ar[h]
                for m in range(KM):
                    part = tp.tile([P, HT], bf, tag="part", name="part", bufs=2)
                    for ch in range(2):
                        ps = pp.tile([P, 512], f32, tag="ps", name="ps", bufs=2)
                        for j in range(JT):
                            nc.tensor.matmul(
                                ps[:], op_s[l][j][:, m * P:(m + 1) * P],
                                c["yg"][j][:, ch * 512:(ch + 1) * 512],
                                start=(j == 0), stop=(j == JT - 1))
                        nc.scalar.activation(
                            part[:, ch * 512:(ch + 1) * 512], ps[:], AF.Copy)
                    nc.sync.dma_start(
                        ar_i[m * P:(m + 1) * P, :], part[:])
                if use_cc:
                    nc.gpsimd.collective_compute(
                        "AllReduce", OP.add, replica_groups=RG,
                        ins=[ar_i.opt()], outs=[ar_o.opt()])

            def emit_resid_loads(h):
                ar_i, ar_o = h_ar[h]
                src = ar_o if use_cc else ar_i
                hds = []
                for m in range(KM):
                    hd = tp.tile([P, HT], bf, tag="hd", name="hd", bufs=2)
                    nc.scalar.dma_start(hd[:], src[m * P:(m + 1) * P, :])
                    hds.append(hd)
                return hds

            def emit_resid_add(h, hds):
                hsl = slice(h * HT, (h + 1) * HT)
                for m in range(KM):
                    nc.vector.tensor_tensor(
                        h_t[m][:, hsl], h_t[m][:, hsl], hds[m][:], OP.add)

            def emit_rfold(l, h):
                """last layer: r = (lin2 @ op_w) @ yg for half h -> rp."""
                c = ctx[(l, h)]
                for ch in range(2):
                    ps = pp.tile([P, 512], f32, tag="ps", name="ps", bufs=2)
                    for j in range(JT):
                        nc.tensor.matmul(
                            ps[0:1, :], w2q_s[:, j:j + 1],
                            c["yg"][j][:, ch * 512:(ch + 1) * 512],
                            start=(j == 0), stop=(j == JT - 1))
                    nc.scalar.activation(
                        rp[:, h * HT + ch * 512:h * HT + (ch + 1) * 512],
                        ps[0:1, :], AF.Copy)

            def emit_lin2h():
                """lin2 applied to pre-final-residual h (overlaps layer 1)."""
                for ch in range(4):
                    ps = pp.tile([P, 512], f32, tag="ps", name="ps", bufs=2)
                    for k in range(KM):
                        nc.tensor.matmul(
                            ps[0:1, :], lin2Tp_s[:, k:k + 1],
                            h_t[k][:, ch * 512:(ch + 1) * 512],
                            start=(k == 0), stop=(k == KM - 1))
                    nc.scalar.activation(
                        l2h[:, ch * 512:(ch + 1) * 512], ps[0:1, :], AF.Copy)

            def emit_final():
                nc.sync.dma_start(ar2_in[:], rp[:])
                if use_cc:
                    nc.gpsimd.collective_compute(
                        "AllReduce", OP.add, replica_groups=RG,
                        ins=[ar2_in.opt()], outs=[ar2_out.opt()])
                arsb = tp.tile([1, T], bf, tag="arsb", name="arsb", bufs=1)
                nc.sync.dma_start(arsb[:], (ar2_out if use_cc else ar2_in)[:])
                nc.vector.tensor_tensor(arsb[:], l2h[:], arsb[:], OP.add)
                yrow = tp.tile([1, T], f32, tag="yrow", name="yrow", bufs=1)
                nc.scalar.activation(yrow[:], arsb[:], AF.Sigmoid, bias=lin2b_s[:])
                nc.sync.dma_start(yrow_d, yrow[:])

            # ---------------- schedule ----------------
            # startup: layer 0 half 0 phase A inline
            emit_rms_act(0, 0)
            emit_rms_hn(0, 0)
            emit_xz_mm(0, 0)
            emit_conv(0, 0)
            emit_xz_fin(0, 0)
            emit_delta(0, 0)
            pre00 = {i: emit_bcast(0, 0, i) for i in range(2)}

            # (0,1) phase A injected under scans(0,0)
            emit_scans(0, 0, pre=pre00, inject={
                1: [lambda: emit_rms_act(0, 1)],
                2: [lambda: emit_rms_hn(0, 1)],
                3: [lambda: emit_xz_mm(0, 1)],
                4: [lambda: emit_conv(0, 1)],
                5: [lambda: emit_xz_fin(0, 1)],
            })
            emit_yg(0, 0)
            emit_outproj(0, 0)
            emit_delta(0, 1)
            pre01 = {i: emit_bcast(0, 1, i) for i in range(2)}

            hd_box = {}
            emit_scans(0, 1, pre=pre01, inject={
                7: [lambda: hd_box.__setitem__(0, emit_resid_loads(0))],
                8: [lambda: emit_resid_add(0, hd_box[0]),
                    lambda: emit_rms_act(1, 0)],
                9: [lambda: emit_rms_hn(1, 0)],
                10: [lambda: emit_xz_mm(1, 0)],
                11: [lambda: emit_conv(1, 0)],
                12: [lambda: emit_xz_fin(1, 0)],
            })
            emit_yg(0, 1)
            emit_outproj(0, 1)
            emit_delta(1, 0)
            pre10 = {i: emit_bcast(1, 0, i) for i in range(2)}

            emit_scans(1, 0, pre=pre10, inject={
                7: [lambda: hd_box.__setitem__(1, emit_resid_loads(1))],
                8: [lambda: emit_resid_add(1, hd_box[1]),
                    lambda: emit_rms_act(1, 1)],
                9: [lambda: emit_rms_hn(1, 1)],
                10: [lambda: emit_xz_mm(1, 1)],
                11: [lambda: emit_conv(1, 1)],
                12: [lambda: emit_xz_fin(1, 1)],
            })
            emit_yg(1, 0)
            emit_rfold(1, 0)
            emit_delta(1, 1)
            pre11 = {i: emit_bcast(1, 1, i) for i in range(2)}

            emit_scans(1, 1, pre=pre11, inject={
                3: [emit_lin2h],
            })
            emit_yg(1, 1)
            emit_rfold(1, 1)
            emit_final()

    nc.compile()
    _CACHE[key] = nc
    return nc


def _prep_inputs(inputs):
    """Host-side prep: per-core input maps (own-quarter TP split)."""
    f32 = np.float32
    x = np.asarray(inputs["x"], f32)
    lin1_w = np.asarray(inputs["lin1_w"], f32)
    lin1_b = np.asarray(inputs["lin1_b"], f32)
    lin2_w = np.asarray(inputs["lin2_w"], f32)
    lin2_b = np.asarray(inputs["lin2_b"], f32)
    norm_w = np.asarray(inputs["norm_w"], f32)
    in_proj_w = np.asarray(inputs["in_proj_w"], f32)
    conv_w = np.asarray(inputs["conv_w"], f32)
    conv_b = np.asarray(inputs["conv_b"], f32)
    x_proj_w = np.asarray(inputs["x_proj_w"], f32)
    dt_proj_w = np.asarray(inputs["dt_proj_w"], f32)
    dt_proj_b = np.asarray(inputs["dt_proj_b"], f32)
    A_log = np.asarray(inputs["A_log"], f32)
    D_param = np.asarray(inputs["D_param"], f32)
    out_proj_w = np.asarray(inputs["out_proj_w"], f32)

    A = -np.exp(A_log)
    import ml_dtypes
    bf = ml_dtypes.bfloat16

    def b16(a):
        return np.ascontiguousarray(a).astype(bf)

    in_maps = []
    for c in range(N_CORES):
        b = c // 4
        q = c % 4
        sh = slice(q * QUART, (q + 1) * QUART)

        m = {}
        m["xT"] = b16(x[b].T)
        m["lin1T"] = b16(lin1_w.T)
        m["lin1b"] = np.ascontiguousarray(
            lin1_b.reshape(KM, P).T).astype(f32)
        m["lin2Tp"] = b16(lin2_w[0].reshape(KM, P).T)
        m["lin2b"] = lin2_b.reshape(1, 1).astype(f32)
        m["idn"] = b16(np.eye(P))

        for l in range(N_LAYERS):
            wn = in_proj_w[l] * norm_w[l][None, :]
            m[f"ipxq{l}"] = b16(wn[:D_INNER][sh].T)               # [512, 256]
            m[f"ipz{l}"] = b16(wn[D_INNER:][sh].T)                # [512, 256]

            cw = conv_w[l, :, 0, :][sh]                           # [256, 4]
            m[f"convw{l}"] = np.ascontiguousarray(
                cw.reshape(JT, P, D_CONV).transpose(1, 0, 2).reshape(P, JT * D_CONV)
            ).astype(f32)
            m[f"convb{l}"] = np.ascontiguousarray(
                conv_b[l][sh].reshape(JT, P).T).astype(f32)

            m[f"xpq{l}"] = b16(x_proj_w[l].T[sh])                 # [256, 64]
            m[f"dtw{l}"] = b16(dt_proj_w[l, sh].T)                # [32, 256]
            m[f"dtb{l}"] = np.ascontiguousarray(
                dt_proj_b[l, sh].reshape(JT, P).T).astype(f32)
            m[f"asc{l}"] = np.ascontiguousarray(
                A[l, sh].reshape(JT, P, D_STATE).transpose(1, 0, 2)
                .reshape(P, JT * D_STATE)).astype(f32)
            dD = np.zeros((P, JT * P), np.float32)
            for j in range(JT):
                dD[:, j * P:(j + 1) * P] = np.diag(
                    D_param[l, sh][j * P:(j + 1) * P])
            m[f"diagD{l}"] = b16(dD)
            m[f"op{l}"] = b16(out_proj_w[l][:, sh].T)             # [256, 512]
        m["w2q"] = b16((lin2_w[0:1, :] @ out_proj_w[N_LAYERS - 1][:, sh]).T)
        in_maps.append(m)
    return in_maps


def kernel(**inputs):
    nc = _build_program()
    in_maps = _prep_inputs(inputs)
    res = run_bass_kernel_spmd(nc, in_maps, core_ids=list(range(N_CORES)))
    out = np.zeros((B, L), np.float32)
    for b in range(B):
        out[b] = res.results[b * 4]["yrow"][0]
    return out


if __name__ == "__main__":
    import reference
    inp = reference.setup_inputs()
    exp = np.asarray(reference.reference(**inp))
    act = kernel(**{k: np.asarray(v) for k, v in inp.items()})
    err = np.abs(act - exp).max() / (np.abs(exp).max() + 1e-12)
    print("max abs err:", np.abs(act - exp).max(), "rel:", err)


# revision 6
# speedup vs baseline: 1.0865x; 1.0045x over previous
"""Mamba-2-layer net on 8 trn2 NeuronCores — v2.

Sharding: core c -> batch b = c // 4, d_inner quarter q = c % 4 (256 ch).
v2 changes vs baseline:
  - TP-split x-path: each core computes in_proj-x/conv/x_proj only for its
    own quarter; a small f32 AllReduce produces dbc (was: 4x-replicated).
  - Time-halved pipeline (HT=1024): out_proj -> AllReduce -> residual ->
    next-layer phase A run per half and are emitted *inside* the other
    half's scan loop so PE/ACT/collective work hides under the DVE scans.
  - Scans chained across halves via per-(n,j) carry columns.
  - Silu ops batched to limit ACT table loads.
"""

import sys
import numpy as np

sys.path.insert(0, "/opt/trn_rl_repo")

import concourse.bass as bass
import concourse.bacc as bacc
import concourse.tile as tile
import concourse.mybir as mybir
from concourse.bass_utils import run_bass_kernel_spmd

dt = mybir.dt
AF = mybir.ActivationFunctionType
OP = mybir.AluOpType

# model dims
B, L = 2, 2048
IN_DIM = 16
D_MODEL = 512
D_INNER = 1024
D_STATE = 16
D_CONV = 4
DT_RANK = 32
N_LAYERS = 2
EPS = 1e-5

# sharding
N_CORES = 8
QUART = D_INNER // 4      # 256 channels per core
T = L
HT = T // 2               # half-T pipeline granularity
P = 128
JT = QUART // P           # 2 d-tiles per quarter
KM = D_MODEL // P         # 4 k-tiles over d_model
PAD = 4
RG = [[0, 1, 2, 3], [4, 5, 6, 7]]

_CACHE = {}


def _build_program(reps=1, use_cc=True, single_core=False):
    key = ("prog2", reps, use_cc, single_core)
    if key in _CACHE:
        return _CACHE[key]

    nc = bacc.Bacc(
        "TRN2",
        target_bir_lowering=False,
        debug=False,
        enable_asserts=False,
        num_devices=1 if single_core else N_CORES,
    )

    bf = dt.bfloat16
    f32 = dt.float32

    # ---------------- DRAM I/O ----------------
    xT = nc.dram_tensor("xT", [IN_DIM, T], bf, kind="ExternalInput").ap()
    lin1T = nc.dram_tensor("lin1T", [IN_DIM, D_MODEL], bf, kind="ExternalInput").ap()
    lin1b = nc.dram_tensor("lin1b", [P, KM], f32, kind="ExternalInput").ap()
    lin2Tp = nc.dram_tensor("lin2Tp", [P, KM], bf, kind="ExternalInput").ap()
    lin2b = nc.dram_tensor("lin2b", [1, 1], f32, kind="ExternalInput").ap()
    idn_d = nc.dram_tensor("idn", [P, P], bf, kind="ExternalInput").ap()

    ipxq_d, ipz_d, convw_d, convb_d, xpq_d, dtw_d, dtb_d, asc_d, dp_d, op_d = (
        [], [], [], [], [], [], [], [], [], [])
    for l in range(N_LAYERS):
        ipxq_d.append(nc.dram_tensor(f"ipxq{l}", [D_MODEL, QUART], bf, kind="ExternalInput").ap())
        ipz_d.append(nc.dram_tensor(f"ipz{l}", [D_MODEL, QUART], bf, kind="ExternalInput").ap())
        convw_d.append(nc.dram_tensor(f"convw{l}", [P, JT * D_CONV], f32, kind="ExternalInput").ap())
        convb_d.append(nc.dram_tensor(f"convb{l}", [P, JT], f32, kind="ExternalInput").ap())
        xpq_d.append(nc.dram_tensor(f"xpq{l}", [QUART, DT_RANK + 2 * D_STATE], bf, kind="ExternalInput").ap())
        dtw_d.append(nc.dram_tensor(f"dtw{l}", [DT_RANK, QUART], bf, kind="ExternalInput").ap())
        dtb_d.append(nc.dram_tensor(f"dtb{l}", [P, JT], f32, kind="ExternalInput").ap())
        asc_d.append(nc.dram_tensor(f"asc{l}", [P, JT * D_STATE], f32, kind="ExternalInput").ap())
        dp_d.append(nc.dram_tensor(f"diagD{l}", [P, JT * P], bf, kind="ExternalInput").ap())
        op_d.append(nc.dram_tensor(f"op{l}", [QUART, D_MODEL], bf, kind="ExternalInput").ap())

    w2q_d = nc.dram_tensor("w2q", [QUART, 1], bf, kind="ExternalInput").ap()
    yrow_d = nc.dram_tensor("yrow", [1, T], f32, kind="ExternalOutput").ap()

    NDBC = DT_RANK + 2 * D_STATE  # 64

    with tile.TileContext(nc) as tc:
        with (
            tc.tile_pool(name="wp", bufs=1) as wp,
            tc.tile_pool(name="tp", bufs=1) as tp,
            tc.tile_pool(name="scp", bufs=1) as scp,
            tc.tile_pool(name="bcp", bufs=1) as bcp,
            tc.tile_pool(name="pp", bufs=1, space="PSUM") as pp,
            tc.tile_pool(name="dramp", bufs=1, space="DRAM") as dramp,
        ):
            # ---------------- weight loads ----------------
            xT_s = wp.tile([IN_DIM, T], bf, tag="xT", name="xT")
            nc.sync.dma_start(xT_s[:], xT)
            lin1T_s = wp.tile([IN_DIM, D_MODEL], bf, tag="lin1T", name="lin1T")
            nc.sync.dma_start(lin1T_s[:], lin1T)
            lin1b_s = wp.tile([P, KM], f32, tag="lin1b", name="lin1b")
            nc.sync.dma_start(lin1b_s[:], lin1b)
            idn_s = wp.tile([P, P], bf, tag="idn", name="idn")
            nc.sync.dma_start(idn_s[:], idn_d)
            lin2Tp_s = wp.tile([P, KM], bf, tag="lin2Tp", name="lin2Tp")
            nc.gpsimd.dma_start(lin2Tp_s[:], lin2Tp)
            lin2b_s = wp.tile([1, 1], f32, tag="lin2b", name="lin2b")
            nc.gpsimd.dma_start(lin2b_s[:], lin2b)
            w2q_s = wp.tile([P, JT], bf, tag="w2q", name="w2q")
            nc.gpsimd.dma_start(
                w2q_s[:], w2q_d.rearrange("(j p) one -> p (j one)", p=P))

            ones1 = wp.tile([1, P], bf, tag="ones1", name="ones1")
            nc.vector.memset(ones1[:], 1.0)
            onesk = wp.tile([P, 1], bf, tag="onesk", name="onesk")
            nc.vector.memset(onesk[:], 1.0)
            zconst = wp.tile([P, 1], f32, tag="zconst", name="zconst")
            nc.vector.memset(zconst[:], 0.0)
            nc.const_aps.aps[(dt.float32, 0.0)] = zconst
            epsconst = wp.tile([P, 1], f32, tag="epsconst", name="epsconst")
            nc.vector.memset(epsconst[:], EPS)
            nc.const_aps.aps[(dt.float32, EPS)] = epsconst
            oneconst = wp.tile([P, 1], f32, tag="oneconst", name="oneconst")
            nc.vector.memset(oneconst[:], 1.0)
            nc.const_aps.aps[(dt.float32, 1.0)] = oneconst

            ipxq_s, ipz_s, convw_s, convb_s, xpq_s, dtw_s, dtb_s, asc_s, dp_s, op_s = (
                [], [], [], [], [], [], [], [], [], [])
            for l in range(N_LAYERS):
                q0 = nc.sync if l == 0 else nc.gpsimd
                t_ = [wp.tile([P, QUART], bf, tag=f"ipxq{l}_{k}", name=f"ipxq{l}_{k}") for k in range(KM)]
                for k in range(KM):
                    q0.dma_start(t_[k][:], ipxq_d[l][k * P:(k + 1) * P, :])
                ipxq_s.append(t_)
                t_ = [wp.tile([P, QUART], bf, tag=f"ipz{l}_{k}", name=f"ipz{l}_{k}") for k in range(KM)]
                for k in range(KM):
                    q0.dma_start(t_[k][:], ipz_d[l][k * P:(k + 1) * P, :])
                ipz_s.append(t_)
                t_ = wp.tile([P, JT * D_CONV], f32, tag=f"convw{l}", name=f"convw{l}")
                q0.dma_start(t_[:], convw_d[l])
                convw_s.append(t_)
                t_ = wp.tile([P, JT], f32, tag=f"convb{l}", name=f"convb{l}")
                q0.dma_start(t_[:], convb_d[l])
                convb_s.append(t_)
                t_ = [wp.tile([P, NDBC], bf, tag=f"xpq{l}_{g}", name=f"xpq{l}_{g}") for g in range(JT)]
                for g in range(JT):
                    q0.dma_start(t_[g][:], xpq_d[l][g * P:(g + 1) * P, :])
                xpq_s.append(t_)
                t_ = wp.tile([DT_RANK, QUART], bf, tag=f"dtw{l}", name=f"dtw{l}")
                q0.dma_start(t_[:], dtw_d[l])
                dtw_s.append(t_)
                t_ = wp.tile([P, JT], f32, tag=f"dtb{l}", name=f"dtb{l}")
                q0.dma_start(t_[:], dtb_d[l])
                dtb_s.append(t_)
                t_ = wp.tile([P, JT * D_STATE], f32, tag=f"asc{l}", name=f"asc{l}")
                q0.dma_start(t_[:], asc_d[l])
                asc_s.append(t_)
                t_ = wp.tile([P, JT * P], bf, tag=f"diagD{l}", name=f"diagD{l}")
                q0.dma_start(t_[:], dp_d[l])
                dp_s.append(t_)
                t_ = [wp.tile([P, D_MODEL], bf, tag=f"op{l}_{j}", name=f"op{l}_{j}") for j in range(JT)]
                for j in range(JT):
                    nc.gpsimd.dma_start(t_[j][:], op_d[l][j * P:(j + 1) * P, :])
                op_s.append(t_)

            # ---------------- persistent tiles ----------------
            h_t = [wp.tile([P, T], bf, tag=f"h{m}", name=f"h{m}") for m in range(KM)]
            hn = [wp.tile([P, T], bf, tag=f"hn{m}", name=f"hn{m}") for m in range(KM)]
            xpad = [[wp.tile([P, PAD + T], bf, tag=f"xpad{l}_{g}", name=f"xpad{l}_{g}")
                     for g in range(JT)] for l in range(N_LAYERS)]
            carry = [wp.tile([P, D_STATE], bf, tag=f"carry{j}", name=f"carry{j}")
                     for j in range(JT)]
            rp = wp.tile([1, T], bf, tag="rp", name="rp")
            l2h = wp.tile([1, T], bf, tag="l2h", name="l2h")

            # DRAM AR buffers
            dbc_ar = [[(dramp.tile([NDBC, HT], bf, tag=f"dbci{l}_{h}", name=f"dbci{l}_{h}"),
                        dramp.tile([NDBC, HT], bf, tag=f"dbco{l}_{h}", name=f"dbco{l}_{h}"))
                       for h in range(2)] for l in range(N_LAYERS)]
            h_ar = [(dramp.tile([D_MODEL, HT], bf, tag=f"ari{h}", name=f"ari{h}"),
                     dramp.tile([D_MODEL, HT], bf, tag=f"aro{h}", name=f"aro{h}"))
                    for h in range(2)]
            ar2_in = dramp.tile([1, T], bf, tag="ar2in", name="ar2in")
            ar2_out = dramp.tile([1, T], bf, tag="ar2out", name="ar2out")

            # per-(l,h) context (transient tile handles)
            ctx = {}

            # ---------------- lin1 ----------------
            for ch in range(4):
                for m in range(KM):
                    ps = pp.tile([P, 512], f32, tag="ps", name="ps", bufs=2)
                    nc.tensor.matmul(
                        ps[:], lin1T_s[:, m * P:(m + 1) * P],
                        xT_s[:, ch * 512:(ch + 1) * 512])
                    nc.scalar.activation(
                        h_t[m][:, ch * 512:(ch + 1) * 512], ps[:],
                        AF.Identity, bias=lin1b_s[:, m:m + 1])

            # ---------------- emitters ----------------
            def emit_rms_act(l, h):
                """rmsnorm ACT/PE chain -> invb (no DVE)."""
                hsl = slice(h * HT, (h + 1) * HT)
                ps1 = [pp.tile([P, 512], f32, tag="ps", name="ps", bufs=2)
                       for ch in range(2)]
                for m in range(KM):
                    sq = tp.tile([P, HT], bf, tag="sq", name="sq", bufs=1)
                    nc.scalar.activation(sq[:], h_t[m][:, hsl], AF.Square)
                    for ch in range(2):
                        nc.tensor.matmul(
                            ps1[ch][0:1, :], onesk[:],
                            sq[:, ch * 512:(ch + 1) * 512],
                            start=(m == 0), stop=(m == KM - 1))
                inv1 = tp.tile([1, HT], f32, tag="inv1", name="inv1", bufs=1)
                for ch in range(2):
                    nc.scalar.activation(
                        inv1[:, ch * 512:(ch + 1) * 512], ps1[ch][0:1, :],
                        AF.Ln, scale=1.0 / D_MODEL, bias=EPS)
                inv1b = tp.tile([1, HT], bf, tag="inv1b", name="inv1b", bufs=1)
                nc.scalar.activation(inv1b[:], inv1[:], AF.Exp, scale=-0.5)
                invb = tp.tile([P, HT], bf, tag="invb", name="invb", bufs=1)
                for ch in range(2):
                    psb = pp.tile([P, 512], f32, tag="ps", name="ps", bufs=2)
                    nc.tensor.matmul(
                        psb[:], ones1[:], inv1b[:, ch * 512:(ch + 1) * 512])
                    nc.scalar.activation(
                        invb[:, ch * 512:(ch + 1) * 512], psb[:], AF.Copy)
                ctx[(l, h)] = {"invb": invb}

            def emit_rms_hn(l, h):
                """hn = h * invb (DVE)."""
                hsl = slice(h * HT, (h + 1) * HT)
                invb = ctx[(l, h)]["invb"]
                for m in range(KM):
                    nc.vector.tensor_tensor(
                        hn[m][:, hsl], h_t[m][:, hsl], invb[:], OP.mult)

            def emit_xz_mm(l, h):
                """in_proj x-half matmuls (own quarter) -> xpad (PE+ACT)."""
                if h == 0:
                    for g in range(JT):
                        nc.vector.memset(xpad[l][g][:, 0:PAD], 0.0)
                for g in range(JT):
                    for ch in range(2):
                        ps = pp.tile([P, 512], f32, tag="ps", name="ps", bufs=2)
                        for k in range(KM):
                            nc.tensor.matmul(
                                ps[:],
                                ipxq_s[l][k][:, g * P:(g + 1) * P],
                                hn[k][:, h * HT + ch * 512:h * HT + (ch + 1) * 512],
                                start=(k == 0), stop=(k == KM - 1))
                        nc.scalar.activation(
                            xpad[l][g][:, PAD + h * HT + ch * 512:
                                       PAD + h * HT + (ch + 1) * 512],
                            ps[:], AF.Copy)

            def emit_conv(l, h):
                """depthwise conv taps + adds (DVE only)."""
                conv_acc = []
                for g in range(JT):
                    tp0 = tp.tile([P, HT], bf, tag=f"tp0_{g}", name=f"tp0_{g}", bufs=1)
                    tp1 = tp.tile([P, HT], bf, tag="tp1", name="tp1", bufs=1)
                    tp2 = tp.tile([P, HT], bf, tag="tp2", name="tp2", bufs=1)
                    tp3 = tp.tile([P, HT], bf, tag="tp3", name="tp3", bufs=1)
                    tps = [tp0, tp1, tp2, tp3]
                    for k in range(D_CONV):
                        nc.vector.tensor_scalar(
                            tps[k][:], xpad[l][g][:, 1 + k + h * HT:1 + k + h * HT + HT],
                            convw_s[l][:, g * D_CONV + k:g * D_CONV + k + 1],
                            None, OP.mult)
                    nc.vector.tensor_tensor(tp0[:], tp0[:], tp1[:], OP.add)
                    nc.vector.tensor_tensor(tp2[:], tp2[:], tp3[:], OP.add)
                    nc.vector.tensor_tensor(tp0[:], tp0[:], tp2[:], OP.add)
                    conv_acc.append(tp0)
                ctx[(l, h)]["conv_acc"] = conv_acc

            def emit_xz_fin(l, h):
                """batched silus (conv + z), x_proj partial, dbc AR send."""
                c = ctx[(l, h)]
                xin = [tp.tile([P, HT], bf, tag=f"xin{g}", name=f"xin{g}", bufs=2)
                       for g in range(JT)]
                sz = [tp.tile([P, HT], bf, tag=f"sz{j}", name=f"sz{j}", bufs=2)
                      for j in range(JT)]
                for g in range(JT):
                    nc.scalar.activation(
                        xin[g][:], c["conv_acc"][g][:], AF.Silu,
                        bias=convb_s[l][:, g:g + 1])
                # x_proj partial (contract own 256 channels)
                xps = pp.tile([NDBC, HT], f32, tag="xps", name="xps", bufs=1)
                for g in range(JT):
                    for ch in range(2):
                        nc.tensor.matmul(
                            xps[:, ch * 512:(ch + 1) * 512], xpq_s[l][g][:],
                            xin[g][:, ch * 512:(ch + 1) * 512],
                            start=(g == 0), stop=(g == JT - 1))
                # evict + AR send (before z so the collective fires early)
                dbc_p = tp.tile([NDBC, HT], bf, tag="dbcp", name="dbcp", bufs=1)
                for ch in range(2):
                    nc.scalar.activation(
                        dbc_p[:, ch * 512:(ch + 1) * 512],
                        xps[:, ch * 512:(ch + 1) * 512], AF.Copy)
                ar_i, ar_o = dbc_ar[l][h]
                nc.sync.dma_start(ar_i[:], dbc_p[:])
                if use_cc:
                    nc.gpsimd.collective_compute(
                        "AllReduce", OP.add, replica_groups=RG,
                        ins=[ar_i.opt()], outs=[ar_o.opt()])
                # z-path: mm -> silu per chunk through the ps ring
                for j in range(JT):
                    for ch in range(2):
                        ps = pp.tile([P, 512], f32, tag="ps", name="ps", bufs=2)
                        for k in range(KM):
                            nc.tensor.matmul(
                                ps[:],
                                ipz_s[l][k][:, j * P:(j + 1) * P],
                                hn[k][:, h * HT + ch * 512:h * HT + (ch + 1) * 512],
                                start=(k == 0), stop=(k == KM - 1))
                        nc.scalar.activation(
                            sz[j][:, ch * 512:(ch + 1) * 512], ps[:], AF.Copy)
                c.update({"xin": xin, "sz": sz})

            def emit_delta(l, h):
                """dbc recv, dt_proj, softplus, du, du2."""
                c = ctx[(l, h)]
                ar_i, ar_o = dbc_ar[l][h]
                dbc = tp.tile([DT_RANK, HT], bf, tag="dbc", name="dbc", bufs=2)
                src_ar = (ar_o if use_cc else ar_i)
                nc.sync.dma_start(dbc[:], src_ar[0:DT_RANK, :])
                dbc4 = tp.tile([P, HT], bf, tag="dbc4", name="dbc4", bufs=2)
                for a in range(4):
                    (nc.sync if a % 2 == 0 else nc.scalar).dma_start(
                        dbc4[32 * a:32 * (a + 1), :], src_ar[DT_RANK:NDBC, :])
                delta = [tp.tile([P, HT], bf, tag=f"delta{j}", name=f"delta{j}", bufs=2)
                         for j in range(JT)]
                du = [tp.tile([P, HT], bf, tag=f"du{j}", name=f"du{j}", bufs=2)
                      for j in range(JT)]
                ex = tp.tile([P, 4 * 512], bf, tag="ex", name="ex", bufs=1)
                for j in range(JT):
                    for ch in range(2):
                        psd = pp.tile([P, 512], f32, tag="ps", name="ps", bufs=2)
                        nc.tensor.matmul(
                            psd[:], dtw_s[l][:, j * P:(j + 1) * P],
                            dbc[:, ch * 512:(ch + 1) * 512])
                        nc.scalar.activation(
                            ex[:, (j * 2 + ch) * 512:(j * 2 + ch + 1) * 512],
                            psd[:], AF.Exp, bias=dtb_s[l][:, j:j + 1])
                for j in range(JT):
                    for ch in range(2):
                        nc.scalar.activation(
                            delta[j][:, ch * 512:(ch + 1) * 512],
                            ex[:, (j * 2 + ch) * 512:(j * 2 + ch + 1) * 512],
                            AF.Ln, bias=1.0)
                for j in range(JT):
                    nc.vector.tensor_tensor(
                        du[j][:], delta[j][:], c["xin"][j][:], OP.mult)
                c.update({"dbc": dbc, "dbc4": dbc4, "delta": delta, "du": du})

            def emit_bcast(l, h, n):
                """B/C row broadcast via doubling DMAs for state n."""
                c = ctx[(l, h)]
                Bb = bcp.tile([P, HT], bf, tag="Bb", name="Bb", bufs=4)
                Cb = bcp.tile([P, HT], bf, tag="Cb", name="Cb", bufs=4)
                cq = nc.gpsimd if n % 2 == 0 else nc.scalar
                d4 = c["dbc4"]
                nc.sync.dma_start(Bb[0:4, :], d4[n:n + 97:32, :])
                cq.dma_start(Cb[0:4, :], d4[D_STATE + n:D_STATE + n + 97:32, :])
                w = 4
                while w < P:
                    nc.sync.dma_start(Bb[w:2 * w, :], Bb[0:w, :])
                    cq.dma_start(Cb[w:2 * w, :], Cb[0:w, :])
                    w *= 2
                return Bb, Cb

            def emit_scans(l, h, inject=None, pre=None):
                """16-state scan loop for half h; inject: {n: [fns]} emitted
                at state n (work that hides under the scans); pre: {n: (Bb, Cb)}
                broadcasts already emitted."""
                inject = inject or {}
                pre = pre or {}
                c = ctx[(l, h)]
                ypsum = [pp.tile([P, HT], f32, tag=f"ypsum{j}", name=f"ypsum{j}", bufs=1)
                         for j in range(JT)]
                c["ypsum"] = ypsum
                for j in range(JT):
                    for ch in range(2):
                        nc.tensor.matmul(
                            ypsum[j][:, ch * 512:(ch + 1) * 512],
                            dp_s[l][:, j * P:(j + 1) * P],
                            c["xin"][j][:, ch * 512:(ch + 1) * 512],
                            start=True, stop=False)
                for n in range(D_STATE):
                    for fn in inject.get(n, []):
                        fn()
                    if n in pre:
                        Bb, Cb = pre[n]
                    else:
                        Bb, Cb = emit_bcast(l, h, n)
                    Bb = Bb[:]
                    Cb = Cb[:]
                    for j in range(JT):
                        dA = scp.tile([P, HT], bf, tag="dA", name="dA", bufs=2)
                        nc.scalar.activation(
                            dA[:], c["delta"][j][:], AF.Exp,
                            scale=asc_s[l][:, j * D_STATE + n:j * D_STATE + n + 1])
                        bx = scp.tile([P, HT], bf, tag="bx", name="bx", bufs=2)
                        nc.vector.tensor_tensor(bx[:], c["du"][j][:], Bb, OP.mult)
                        hs = scp.tile([P, HT], bf, tag="hs", name="hs", bufs=2)
                        init = 0.0 if h == 0 else carry[j][:, n:n + 1]
                        nc.vector.tensor_tensor_scan(
                            hs[:], dA[:], bx[:], init, OP.mult, OP.add)
                        if h == 0:
                            nc.vector.tensor_copy(
                                carry[j][:, n:n + 1], hs[:, HT - 1:HT])
                        hc = scp.tile([P, HT], bf, tag="hc", name="hc", bufs=2)
                        nc.vector.tensor_tensor(hc[:], hs[:], Cb, OP.mult)
                        for ch in range(2):
                            nc.tensor.matmul(
                                ypsum[j][:, ch * 512:(ch + 1) * 512],
                                idn_s[:], hc[:, ch * 512:(ch + 1) * 512],
                                start=False, stop=(n == D_STATE - 1))

            def emit_yg(l, h):
                """ypsum -> gated yg (evicts ypsum psum)."""
                c = ctx[(l, h)]
                yg = [tp.tile([P, HT], bf, tag=f"yg{j}", name=f"yg{j}", bufs=1)
                      for j in range(JT)]
                t1s = []
                for j in range(JT):
                    t1 = tp.tile([P, HT], bf, tag=f"t1_{j}", name=f"t1_{j}", bufs=1)
                    for ch in range(2):
                        nc.scalar.activation(
                            t1[:, ch * 512:(ch + 1) * 512],
                            c["ypsum"][j][:, ch * 512:(ch + 1) * 512], AF.Copy)
                    t1s.append(t1)
                for j in range(JT):
                    nc.scalar.activation(
                        c["sz"][j][:], c["sz"][j][:], AF.Silu)
                for j in range(JT):
                    nc.vector.tensor_tensor(yg[j][:], t1s[j][:], c["sz"][j][:], OP.mult)
                c["yg"] = yg

            def emit_outproj(l, h):
                """out_proj partial + AR send (layer 0 only)."""
                c = ctx[(l, h)]
                ar_i, ar_o = h_ar[h]
                for m in range(KM):
                    part = tp.tile([P, HT], bf, tag="part", name="part", bufs=2)
                    for ch in range(2):
                        ps = pp.tile([P, 512], f32, tag="ps", name="ps", bufs=2)
                        for j in range(JT):
                            nc.tensor.matmul(
                                ps[:], op_s[l][j][:, m * P:(m + 1) * P],
                                c["yg"][j][:, ch * 512:(ch + 1) * 512],
                                start=(j == 0), stop=(j == JT - 1))
                        nc.scalar.activation(
                            part[:, ch * 512:(ch + 1) * 512], ps[:], AF.Copy)
                    nc.sync.dma_start(
                        ar_i[m * P:(m + 1) * P, :], part[:])
                if use_cc:
                    nc.gpsimd.collective_compute(
                        "AllReduce", OP.add, replica_groups=RG,
                        ins=[ar_i.opt()], outs=[ar_o.opt()])

            def emit_resid_loads(h):
                ar_i, ar_o = h_ar[h]
                src = ar_o if use_cc else ar_i
                hds = []
                for m in range(KM):
                    hd = tp.tile([P, HT], bf, tag="hd", name="hd", bufs=2)
                    nc.scalar.dma_start(hd[:], src[m * P:(m + 1) * P, :])
                    hds.append(hd)
                return hds

            def emit_resid_add(h, hds):
                hsl = slice(h * HT, (h + 1) * HT)
                for m in range(KM):
                    nc.vector.tensor_tensor(
                        h_t[m][:, hsl], h_t[m][:, hsl], hds[m][:], OP.add)

            def emit_rfold(l, h):
                """last layer: r = (lin2 @ op_w) @ yg for half h -> rp."""
                c = ctx[(l, h)]
                for ch in range(2):
                    ps = pp.tile([P, 512], f32, tag="ps", name="ps", bufs=2)
                    for j in range(JT):
                        nc.tensor.matmul(
                            ps[0:1, :], w2q_s[:, j:j + 1],
                            c["yg"][j][:, ch * 512:(ch + 1) * 512],
                            start=(j == 0), stop=(j == JT - 1))
                    nc.scalar.activation(
                        rp[:, h * HT + ch * 512:h * HT + (ch + 1) * 512],
                        ps[0:1, :], AF.Copy)

            def emit_lin2h():
                """lin2 applied to pre-final-residual h (overlaps layer 1)."""
                for ch in range(4):
                    ps = pp.tile([P, 512], f32, tag="ps", name="ps", bufs=2)
                    for k in range(KM):
                        nc.tensor.matmul(
                            ps[0:1, :], lin2Tp_s[:, k:k + 1],
                            h_t[k][:, ch * 512:(ch + 1) * 512],
                            start=(k == 0), stop=(k == KM - 1))
                    nc.scalar.activation(
                        l2h[:, ch * 512:(ch + 1) * 512], ps[0:1, :], AF.Copy)

            def emit_final():
                nc.sync.dma_start(ar2_in[:], rp[:])
                if use_cc:
                    nc.gpsimd.collective_compute(
                        "AllReduce", OP.add, replica_groups=RG,
                        ins=[ar2_in.opt()], outs=[ar2_out.opt()])
                arsb = tp.tile([1, T], bf, tag="arsb", name="arsb", bufs=1)
                nc.sync.dma_start(arsb[:], (ar2_out if use_cc else ar2_in)[:])
                nc.vector.tensor_tensor(arsb[:], l2h[:], arsb[:], OP.add)
                yrow = tp.tile([1, T], f32, tag="yrow", name="yrow", bufs=1)
                nc.scalar.activation(yrow[:], arsb[:], AF.Sigmoid, bias=lin2b_s[:])
                nc.sync.dma_start(yrow_d, yrow[:])

            # ---------------- schedule ----------------
            # startup: layer 0 half 0 phase A inline
            emit_rms_act(0, 0)
            emit_rms_hn(0, 0)
            emit_xz_mm(0, 0)
            emit_conv(0, 0)
            emit_xz_fin(0, 0)
            emit_delta(0, 0)
            pre00 = {i: emit_bcast(0, 0, i) for i in range(2)}

            # (0,1) phase A injected under scans(0,0)
            emit_scans(0, 0, pre=pre00, inject={
                1: [lambda: emit_rms_act(0, 1)],
                2: [lambda: emit_rms_hn(0, 1)],
                3: [lambda: emit_xz_mm(0, 1)],
                4: [lambda: emit_conv(0, 1)],
                5: [lambda: emit_xz_fin(0, 1)],
            })
            emit_yg(0, 0)
            emit_outproj(0, 0)
            emit_delta(0, 1)
            pre01 = {i: emit_bcast(0, 1, i) for i in range(2)}

            hd_box = {}
            emit_scans(0, 1, pre=pre01, inject={
                7: [lambda: hd_box.__setitem__(0, emit_resid_loads(0))],
                8: [lambda: emit_resid_add(0, hd_box[0]),
                    lambda: emit_rms_act(1, 0)],
                9: [lambda: emit_rms_hn(1, 0)],
                10: [lambda: emit_xz_mm(1, 0)],
                11: [lambda: emit_conv(1, 0)],
                12: [lambda: emit_xz_fin(1, 0)],
            })
            emit_yg(0, 1)
            emit_outproj(0, 1)
            emit_delta(1, 0)
            pre10 = {i: emit_bcast(1, 0, i) for i in range(2)}

            emit_scans(1, 0, pre=pre10, inject={
                7: [lambda: hd_box.__setitem__(1, emit_resid_loads(1))],
                8: [lambda: emit_resid_add(1, hd_box[1]),
                    lambda: emit_rms_act(1, 1)],
                9: [lambda: emit_rms_hn(1, 1)],
                10: [lambda: emit_xz_mm(1, 1)],
                11: [lambda: emit_conv(1, 1)],
                12: [lambda: emit_xz_fin(1, 1)],
            })
            emit_yg(1, 0)
            emit_rfold(1, 0)
            emit_delta(1, 1)
            pre11 = {i: emit_bcast(1, 1, i) for i in range(2)}

            emit_scans(1, 1, pre=pre11, inject={
                3: [emit_lin2h],
            })
            emit_yg(1, 1)
            emit_rfold(1, 1)
            emit_final()

    nc.compile()
    _CACHE[key] = nc
    return nc


def _prep_inputs(inputs):
    """Host-side prep: per-core input maps (own-quarter TP split)."""
    f32 = np.float32
    x = np.asarray(inputs["x"], f32)
    lin1_w = np.asarray(inputs["lin1_w"], f32)
    lin1_b = np.asarray(inputs["lin1_b"], f32)
    lin2_w = np.asarray(inputs["lin2_w"], f32)
    lin2_b = np.asarray(inputs["lin2_b"], f32)
    norm_w = np.asarray(inputs["norm_w"], f32)
    in_proj_w = np.asarray(inputs["in_proj_w"], f32)
    conv_w = np.asarray(inputs["conv_w"], f32)
    conv_b = np.asarray(inputs["conv_b"], f32)
    x_proj_w = np.asarray(inputs["x_proj_w"], f32)
    dt_proj_w = np.asarray(inputs["dt_proj_w"], f32)
    dt_proj_b = np.asarray(inputs["dt_proj_b"], f32)
    A_log = np.asarray(inputs["A_log"], f32)
    D_param = np.asarray(inputs["D_param"], f32)
    out_proj_w = np.asarray(inputs["out_proj_w"], f32)

    A = -np.exp(A_log)
    import ml_dtypes
    bf = ml_dtypes.bfloat16

    def b16(a):
        return np.ascontiguousarray(a).astype(bf)

    in_maps = []
    for c in range(N_CORES):
        b = c // 4
        q = c % 4
        sh = slice(q * QUART, (q + 1) * QUART)

        m = {}
        m["xT"] = b16(x[b].T)
        m["lin1T"] = b16(lin1_w.T)
        m["lin1b"] = np.ascontiguousarray(
            lin1_b.reshape(KM, P).T).astype(f32)
        m["lin2Tp"] = b16(lin2_w[0].reshape(KM, P).T)
        m["lin2b"] = lin2_b.reshape(1, 1).astype(f32)
        m["idn"] = b16(np.eye(P))

        for l in range(N_LAYERS):
            wn = in_proj_w[l] * norm_w[l][None, :]
            m[f"ipxq{l}"] = b16(wn[:D_INNER][sh].T)               # [512, 256]
            m[f"ipz{l}"] = b16(wn[D_INNER:][sh].T)                # [512, 256]

            cw = conv_w[l, :, 0, :][sh]                           # [256, 4]
            m[f"convw{l}"] = np.ascontiguousarray(
                cw.reshape(JT, P, D_CONV).transpose(1, 0, 2).reshape(P, JT * D_CONV)
            ).astype(f32)
            m[f"convb{l}"] = np.ascontiguousarray(
                conv_b[l][sh].reshape(JT, P).T).astype(f32)

            m[f"xpq{l}"] = b16(x_proj_w[l].T[sh])                 # [256, 64]
            m[f"dtw{l}"] = b16(dt_proj_w[l, sh].T)                # [32, 256]
            m[f"dtb{l}"] = np.ascontiguousarray(
                dt_proj_b[l, sh].reshape(JT, P).T).astype(f32)
            m[f"asc{l}"] = np.ascontiguousarray(
                A[l, sh].reshape(JT, P, D_STATE).transpose(1, 0, 2)
                .reshape(P, JT * D_STATE)).astype(f32)
            dD = np.zeros((P, JT * P), np.float32)
            for j in range(JT):
                dD[:, j * P:(j + 1) * P] = np.diag(
                    D_param[l, sh][j * P:(j + 1) * P])
            m[f"diagD{l}"] = b16(dD)
            m[f"op{l}"] = b16(out_proj_w[l][:, sh].T)             # [256, 512]
        m["w2q"] = b16((lin2_w[0:1, :] @ out_proj_w[N_LAYERS - 1][:, sh]).T)
        in_maps.append(m)
    return in_maps


def kernel(**inputs):
    nc = _build_program()
    in_maps = _prep_inputs(inputs)
    res = run_bass_kernel_spmd(nc, in_maps, core_ids=list(range(N_CORES)))
    out = np.zeros((B, L), np.float32)
    for b in range(B):
        out[b] = res.results[b * 4]["yrow"][0]
    return out


if __name__ == "__main__":
    import reference
    inp = reference.setup_inputs()
    exp = np.asarray(reference.reference(**inp))
    act = kernel(**{k: np.asarray(v) for k, v in inp.items()})
    err = np.abs(act - exp).max() / (np.abs(exp).max() + 1e-12)
    print("max abs err:", np.abs(act - exp).max(), "rel:", err)
